# revision 1
# baseline (speedup 1.0000x reference)
"""HaarWavelet2D (level=2) Trainium2 kernel.

Contract: kernel(x, level) with x [8, 64, 256, 256] fp32, level=2.
Returns (low_freq, high_freq), each [8, 64, 256, 256] fp32 — matching the
jax reference (2-level Haar decomposition with bilinear resizes).

Sharding: data-parallel over the batch dim — core b processes x[b] (64
channels). Inside each core, channels are processed in groups of G=2 with
rows in partitions (even/odd row-parity tiles), columns*channels in the
free dimension.

Math (validated vs reference in model.py):
  s = x[:,j]+x[:,j+1]; d = x[:,j]-x[:,j+1]
  t1 = s[r]-s[r+1];   m = max(|d[r]|,|d[r+1]|)       (abs_max ALU op)
  ch0 = 0.5|t1| + m   (== 2*(|lh|+|hl|+|hh|) of level 0)
  Y_L = Va @ s        (Va = 0.25*V255@Sv1 — vertical resize+pair-sum fold)
  Y_h = (0.25*V255) @ ch0
  L0 = Rh255(Y_L); h0 = Rh255(Y_h)                   (horizontal resize)
  level 1 on L0 via stride-2 column pairs + row-parity tiles, V128 resize
  high = h0 + h1; low = Rh128(V128q @ lsum1)
All vertical linear ops run on the tensor engine as banded-matrix matmuls
(bf16 weights, fp32 PSUM); horizontal resizes use the pad+diff trick on
the vector engine; scalar/gpsimd engines do casts and shifted copies.
"""

import sys

if "/opt/trn_rl_repo" not in sys.path:
    sys.path.insert(0, "/opt/trn_rl_repo")

import numpy as np
import ml_dtypes

BF = ml_dtypes.bfloat16

B_, C_, H_, W_ = 8, 64, 256, 256
NCORES = 8
G = 2  # channels per inner iteration


# ----------------------------------------------------------------------------
# host-side weight construction
# ----------------------------------------------------------------------------

def _resize_matrix(n, N):
    M = np.zeros((N, n), dtype=np.float64)
    for i in range(N):
        c = (i + 0.5) * n / N - 0.5
        j0 = int(np.floor(c))
        f = c - j0
        M[i, min(max(j0, 0), n - 1)] += 1.0 - f
        M[i, min(max(j0 + 1, 0), n - 1)] += f
    return M


def _build_weights():
    V255 = _resize_matrix(255, 256)
    V128 = _resize_matrix(128, 256)
    Sv1 = np.zeros((255, 256))
    for r in range(255):
        Sv1[r, r] = 1.0
        Sv1[r, r + 1] = 1.0

    Va = 0.25 * (V255 @ Sv1)      # [256, 256]
    V255s = 0.25 * V255           # [256, 255]
    V128q = 0.25 * V128           # [256, 128]
    W0 = np.array([V255[i, i - 1] if i >= 1 else 0.0 for i in range(256)])

    w = {
        # L0 vertical: Y_L(parity p rows) = Va[p::2, 0::2] @ sE + Va[p::2, 1::2] @ sO
        "w_va_ee": Va[0::2, 0::2].T,   # [128,128]
        "w_va_eo": Va[0::2, 1::2].T,
        "w_va_oe": Va[1::2, 0::2].T,
        "w_va_oo": Va[1::2, 1::2].T,
        # h0 vertical: rows 0:128 (A) / 128:256 (B); ch0 rows even(128)/odd(127)
        "w_vh_ae": V255s[0:128, 0::2].T,   # [128,128]
        "w_vh_ao": V255s[0:128, 1::2].T,   # [127,128]
        "w_vh_be": V255s[128:256, 0::2].T,
        "w_vh_bo": V255s[128:256, 1::2].T,
        # level-1 vertical
        "w_vq_a": V128q[0:128, :].T,   # [128,128]
        "w_vq_b": V128q[128:256, :].T,
        # horizontal 255->256 weights, replicated over partitions
        "w0t": np.tile(W0[None, :], (128, 1)),   # [128,256]
    }
    return {k: v.astype(BF) for k, v in w.items()}


_WEIGHTS = None


def _weights():
    global _WEIGHTS
    if _WEIGHTS is None:
        _WEIGHTS = _build_weights()
    return _WEIGHTS


# ----------------------------------------------------------------------------
# bass program
# ----------------------------------------------------------------------------

_NC_CACHE = {}


def build_nc(C=C_):
    key = C
    if key in _NC_CACHE:
        return _NC_CACHE[key]

    import concourse.bass as bass
    import concourse.bacc as bacc
    import concourse.tile as tile
    import concourse.mybir as mybir

    F32 = mybir.dt.float32
    BF16 = mybir.dt.bfloat16
    Alu = mybir.AluOpType
    Act = mybir.ActivationFunctionType
    P = 128

    nc = bacc.Bacc("TRN2", target_bir_lowering=False)
    x_d = nc.dram_tensor("x", [C, H_, W_], F32, kind="ExternalInput")
    wt = _weights()
    w_d = {
        name: nc.dram_tensor(name, list(arr.shape), BF16, kind="ExternalInput")
        for name, arr in wt.items()
    }
    low_d = nc.dram_tensor("low", [C, H_, W_], F32, kind="ExternalOutput")
    high_d = nc.dram_tensor("high", [C, H_, W_], F32, kind="ExternalOutput")

    def bcast_cols(ap, g):
        # weight AP [128, N] -> [128, (0,g), N]: repeat per channel group
        return bass.AP(tensor=ap.tensor, offset=ap.offset,
                       ap=[ap.ap[0], [0, g], ap.ap[1]])

    with tile.TileContext(nc) as tc:
        with (
            tc.tile_pool(name="consts", bufs=1) as consts,
            tc.tile_pool(name="xin", bufs=2) as xin,
            tc.tile_pool(name="sd", bufs=2) as sd,
            tc.tile_pool(name="mid", bufs=2) as mid,
            tc.tile_pool(name="hor", bufs=2) as hor,
            tc.tile_pool(name="lv1", bufs=2) as lv1,
            tc.tile_pool(name="outp", bufs=2) as outp,
            tc.tile_pool(name="ps0", bufs=1, space="PSUM") as ps0,
            tc.tile_pool(name="ps1", bufs=2, space="PSUM") as ps1,
        ):
            wtile = {}
            for name, arr in wt.items():
                t = consts.tile(list(arr.shape), BF16, tag=name)
                nc.sync.dma_start(out=t, in_=w_d[name][:, :])
                wtile[name] = t

            def rh255(Y, out_name):
                """Horizontal 255->256 resize of a merged PSUM tile
                Y [128, 2, G, 256] (cols 0:255 valid in each half).
                Returns bf16 SBUF tile [128, 2, G, 256]."""
                Yv = Y[:, :, 0:G * 255].rearrange("p h (g w) -> p h g w", w=255)
                q = hor.tile([P, 2, G, 256], BF16, tag=f"q_{out_name}")
                nc.scalar.copy(out=q[:, :, :, 0:255], in_=Yv)
                nc.scalar.copy(out=q[:, :, :, 255:256], in_=Yv[:, :, :, 254:255])
                q1 = hor.tile([P, 2, G, 256], BF16, tag=f"q1_{out_name}")
                nc.gpsimd.tensor_copy(out=q1[:, :, :, 1:256], in_=q[:, :, :, 0:255])
                nc.gpsimd.tensor_copy(out=q1[:, :, :, 0:1], in_=q[:, :, :, 0:1])
                diff = hor.tile([P, 2, G, 256], BF16, tag=f"df_{out_name}")
                nc.vector.tensor_tensor(out=diff, in0=q1, in1=q, op=Alu.subtract)
                w0b = wtile["w0t"][:, :]
                w0_ap = bass.AP(tensor=w0b.tensor, offset=w0b.offset,
                                ap=[w0b.ap[0], [0, 2], [0, G], w0b.ap[1]])
                mult = hor.tile([P, 2, G, 256], BF16, tag=f"mu_{out_name}")
                nc.vector.tensor_tensor(out=mult, in0=diff, in1=w0_ap, op=Alu.mult)
                out = hor.tile([P, 2, G, 256], BF16, tag=out_name)
                nc.vector.tensor_tensor(out=out, in0=q, in1=mult, op=Alu.add)
                return out

            n_iter = C // G
            for it in range(n_iter):
                c0 = it * G

                # ---- load x row-parity tiles -------------------------------
                xE = xin.tile([P, G, W_], F32, tag="xE")
                xO = xin.tile([P, G, W_], F32, tag="xO")
                nc.sync.dma_start(
                    out=xE, in_=x_d[c0:c0 + G, 0:H_:2, :].rearrange("c r w -> r c w"))
                nc.sync.dma_start(
                    out=xO, in_=x_d[c0:c0 + G, 1:H_:2, :].rearrange("c r w -> r c w"))

                # ---- level-0 horizontal pair sum/diff ----------------------
                # cast to bf16 first (2x_2P) so s/d run in the 2x_1P TT mode;
                # the +1-column-shifted operand comes from a gpsimd copy so
                # both TT operands stay 4B-aligned
                xbE = sd.tile([P, G, W_], BF16, tag="xbE")
                xbO = sd.tile([P, G, W_], BF16, tag="xbO")
                nc.vector.tensor_copy(out=xbE, in_=xE)
                nc.vector.tensor_copy(out=xbO, in_=xO)
                xbE1 = sd.tile([P, G, 255], BF16, tag="xbE1")
                xbO1 = sd.tile([P, G, 255], BF16, tag="xbO1")
                nc.gpsimd.tensor_copy(out=xbE1, in_=xbE[:, :, 1:256])
                nc.gpsimd.tensor_copy(out=xbO1, in_=xbO[:, :, 1:256])
                sE = sd.tile([P, G, 255], BF16, tag="sE")
                sO = sd.tile([P, G, 255], BF16, tag="sO")
                dE = sd.tile([P, G, 255], BF16, tag="dE")
                dO = sd.tile([P, G, 255], BF16, tag="dO")
                nc.vector.tensor_tensor(out=sE, in0=xbE[:, :, 0:255], in1=xbE1, op=Alu.add)
                nc.vector.tensor_tensor(out=sO, in0=xbO[:, :, 0:255], in1=xbO1, op=Alu.add)
                nc.vector.tensor_tensor(out=dE, in0=xbE[:, :, 0:255], in1=xbE1, op=Alu.subtract)
                nc.vector.tensor_tensor(out=dO, in0=xbO[:, :, 0:255], in1=xbO1, op=Alu.subtract)
                # |d| on the scalar engine (abs_max is not supported by codegen)
                adE = sd.tile([P, G, 255], BF16, tag="adE")
                adO = sd.tile([P, G, 255], BF16, tag="adO")
                nc.scalar.activation(out=adE, in_=dE, func=Act.Abs)
                nc.scalar.activation(out=adO, in_=dO, func=Act.Abs)
                # shifted copies (rows 2,4..254) via SBUF->SBUF DMA
                sE2 = sd.tile([127, G, 255], BF16, tag="sE2")
                adE2 = sd.tile([127, G, 255], BF16, tag="adE2")
                nc.sync.dma_start(out=sE2, in_=sE[1:128, :, :])
                nc.sync.dma_start(out=adE2, in_=adE[1:128, :, :])

                # ---- level-0 vertical pair ops -----------------------------
                t1E = mid.tile([P, G, 255], BF16, tag="t1E")
                t1O = mid.tile([127, G, 255], BF16, tag="t1O")
                mE = mid.tile([P, G, 255], BF16, tag="mE")
                mO = mid.tile([127, G, 255], BF16, tag="mO")
                nc.vector.tensor_tensor(out=t1E, in0=sE, in1=sO, op=Alu.subtract)
                nc.vector.tensor_tensor(out=t1O, in0=sO[0:127, :, :], in1=sE2, op=Alu.subtract)
                nc.vector.tensor_tensor(out=mE, in0=adE, in1=adO, op=Alu.max)
                nc.vector.tensor_tensor(out=mO, in0=adO[0:127, :, :], in1=adE2, op=Alu.max)

                a1E = mid.tile([P, G, 255], BF16, tag="a1E")
                a1O = mid.tile([127, G, 255], BF16, tag="a1O")
                nc.scalar.activation(out=a1E, in_=t1E, func=Act.Abs, scale=0.5)
                nc.scalar.activation(out=a1O, in_=t1O, func=Act.Abs, scale=0.5)
                ch0E = mid.tile([P, G, 255], BF16, tag="ch0E")
                ch0O = mid.tile([127, G, 255], BF16, tag="ch0O")
                nc.vector.tensor_tensor(out=ch0E, in0=a1E, in1=mE, op=Alu.add)
                nc.vector.tensor_tensor(out=ch0O, in0=a1O, in1=mO, op=Alu.add)

                # ---- level-0 vertical matmuls ------------------------------
                # merged PSUM tiles: [:,0] / [:,1] halves are bank-aligned
                # (2048B each); matmuls write cols 0:255 of each half
                NF = G * 255
                Y_L = ps0.tile([P, 2, 512], F32, tag="Y_L")
                Y_h = ps0.tile([P, 2, 512], F32, tag="Y_h")
                nc.tensor.matmul(out=Y_L[:, 0, 0:NF], lhsT=wtile["w_va_ee"][:, :], rhs=sE, start=True, stop=False)
                nc.tensor.matmul(out=Y_L[:, 0, 0:NF], lhsT=wtile["w_va_eo"][:, :], rhs=sO, start=False, stop=True)
                nc.tensor.matmul(out=Y_L[:, 1, 0:NF], lhsT=wtile["w_va_oe"][:, :], rhs=sE, start=True, stop=False)
                nc.tensor.matmul(out=Y_L[:, 1, 0:NF], lhsT=wtile["w_va_oo"][:, :], rhs=sO, start=False, stop=True)
                nc.tensor.matmul(out=Y_h[:, 0, 0:NF], lhsT=wtile["w_vh_ae"][:, :], rhs=ch0E, start=True, stop=False)
                nc.tensor.matmul(out=Y_h[:, 0, 0:NF], lhsT=wtile["w_vh_ao"][:, :], rhs=ch0O, start=False, stop=True)
                nc.tensor.matmul(out=Y_h[:, 1, 0:NF], lhsT=wtile["w_vh_be"][:, :], rhs=ch0E, start=True, stop=False)
                nc.tensor.matmul(out=Y_h[:, 1, 0:NF], lhsT=wtile["w_vh_bo"][:, :], rhs=ch0O, start=False, stop=True)

                # ---- level-0 horizontal resizes ----------------------------
                L0x = rh255(Y_L, "L0x")
                h0x = rh255(Y_h, "h0x")
                L0e, L0o = L0x[:, 0], L0x[:, 1]
                h0A, h0B = h0x[:, 0], h0x[:, 1]

                # ---- level-1 elementwise -----------------------------------
                s2e = lv1.tile([P, G, 128], BF16, tag="s2e")
                s2o = lv1.tile([P, G, 128], BF16, tag="s2o")
                d2e = lv1.tile([P, G, 128], BF16, tag="d2e")
                d2o = lv1.tile([P, G, 128], BF16, tag="d2o")
                nc.vector.tensor_tensor(out=s2e, in0=L0e[:, :, 0:256:2], in1=L0e[:, :, 1:256:2], op=Alu.add)
                nc.vector.tensor_tensor(out=s2o, in0=L0o[:, :, 0:256:2], in1=L0o[:, :, 1:256:2], op=Alu.add)
                nc.vector.tensor_tensor(out=d2e, in0=L0e[:, :, 0:256:2], in1=L0e[:, :, 1:256:2], op=Alu.subtract)
                nc.vector.tensor_tensor(out=d2o, in0=L0o[:, :, 0:256:2], in1=L0o[:, :, 1:256:2], op=Alu.subtract)
                lsum1 = lv1.tile([P, G, 128], BF16, tag="lsum1")
                t1b = lv1.tile([P, G, 128], BF16, tag="t1b")
                ad2e = lv1.tile([P, G, 128], BF16, tag="ad2e")
                ad2o = lv1.tile([P, G, 128], BF16, tag="ad2o")
                m1 = lv1.tile([P, G, 128], BF16, tag="m1")
                nc.vector.tensor_tensor(out=lsum1, in0=s2e, in1=s2o, op=Alu.add)
                nc.vector.tensor_tensor(out=t1b, in0=s2e, in1=s2o, op=Alu.subtract)
                nc.scalar.activation(out=ad2e, in_=d2e, func=Act.Abs)
                nc.scalar.activation(out=ad2o, in_=d2o, func=Act.Abs)
                nc.vector.tensor_tensor(out=m1, in0=ad2e, in1=ad2o, op=Alu.max)
                a1b = lv1.tile([P, G, 128], BF16, tag="a1b")
                nc.scalar.activation(out=a1b, in_=t1b, func=Act.Abs, scale=0.5)
                ch1 = lv1.tile([P, G, 128], BF16, tag="ch1")
                nc.vector.tensor_tensor(out=ch1, in0=a1b, in1=m1, op=Alu.add)

                # ---- level-1 vertical matmuls ------------------------------
                Y_lo = ps1.tile([P, 2, G, 128], F32, tag="Y_lo")
                Y_h1 = ps1.tile([P, 2, G, 128], F32, tag="Y_h1")
                nc.tensor.matmul(out=Y_lo[:, 0], lhsT=wtile["w_vq_a"][:, :], rhs=lsum1, start=True, stop=True)
                nc.tensor.matmul(out=Y_lo[:, 1], lhsT=wtile["w_vq_b"][:, :], rhs=lsum1, start=True, stop=True)
                nc.tensor.matmul(out=Y_h1[:, 0], lhsT=wtile["w_vq_a"][:, :], rhs=ch1, start=True, stop=True)
                nc.tensor.matmul(out=Y_h1[:, 1], lhsT=wtile["w_vq_b"][:, :], rhs=ch1, start=True, stop=True)

                # ---- level-1 horizontal (128->256) + finalization ----------
                def rh128(Y, name):
                    """Y: PSUM [128, 2, G, 128] -> (ev, od) bf16 [128,2,G,128]."""
                    q = lv1.tile([P, 2, G, 129], BF16, tag=f"q_{name}")
                    nc.scalar.copy(out=q[:, :, :, 0:128], in_=Y)
                    nc.scalar.copy(out=q[:, :, :, 128:129], in_=Y[:, :, :, 127:128])
                    q1 = lv1.tile([P, 2, G, 129], BF16, tag=f"q1_{name}")
                    nc.gpsimd.tensor_copy(out=q1[:, :, :, 1:129], in_=q[:, :, :, 0:128])
                    nc.gpsimd.tensor_copy(out=q1[:, :, :, 0:1], in_=q[:, :, :, 0:1])
                    diff = lv1.tile([P, 2, G, 129], BF16, tag=f"df_{name}")
                    nc.vector.tensor_tensor(out=diff, in0=q1, in1=q, op=Alu.subtract)
                    ev = lv1.tile([P, 2, G, 128], BF16, tag=f"ev_{name}")
                    od = lv1.tile([P, 2, G, 128], BF16, tag=f"od_{name}")
                    nc.vector.scalar_tensor_tensor(
                        out=ev, in0=diff[:, :, :, 0:128], scalar=0.25,
                        in1=q[:, :, :, 0:128], op0=Alu.mult, op1=Alu.add)
                    nc.vector.scalar_tensor_tensor(
                        out=od, in0=diff[:, :, :, 1:129], scalar=-0.25,
                        in1=q[:, :, :, 0:128], op0=Alu.mult, op1=Alu.add)
                    return ev, od

                lo_ev, lo_od = rh128(Y_lo, "lo")
                h1_ev, h1_od = rh128(Y_h1, "h1")

                lowA = outp.tile([P, G, W_], F32, tag="lowA")
                lowB = outp.tile([P, G, W_], F32, tag="lowB")
                nc.gpsimd.tensor_copy(out=lowA[:, :, 0:256:2], in_=lo_ev[:, 0])
                nc.gpsimd.tensor_copy(out=lowA[:, :, 1:256:2], in_=lo_od[:, 0])
                nc.gpsimd.tensor_copy(out=lowB[:, :, 0:256:2], in_=lo_ev[:, 1])
                nc.gpsimd.tensor_copy(out=lowB[:, :, 1:256:2], in_=lo_od[:, 1])

                highA = outp.tile([P, G, W_], F32, tag="highA")
                highB = outp.tile([P, G, W_], F32, tag="highB")
                nc.vector.tensor_tensor(out=highA[:, :, 0:256:2], in0=h1_ev[:, 0],
                                        in1=h0A[:, :, 0:256:2], op=Alu.add)
                nc.vector.tensor_tensor(out=highA[:, :, 1:256:2], in0=h1_od[:, 0],
                                        in1=h0A[:, :, 1:256:2], op=Alu.add)
                nc.vector.tensor_tensor(out=highB[:, :, 0:256:2], in0=h1_ev[:, 1],
                                        in1=h0B[:, :, 0:256:2], op=Alu.add)
                nc.vector.tensor_tensor(out=highB[:, :, 1:256:2], in0=h1_od[:, 1],
                                        in1=h0B[:, :, 1:256:2], op=Alu.add)

                # ---- store --------------------------------------------------
                nc.sync.dma_start(
                    out=low_d[c0:c0 + G, 0:128, :].rearrange("c r w -> r c w"), in_=lowA)
                nc.sync.dma_start(
                    out=low_d[c0:c0 + G, 128:256, :].rearrange("c r w -> r c w"), in_=lowB)
                nc.sync.dma_start(
                    out=high_d[c0:c0 + G, 0:128, :].rearrange("c r w -> r c w"), in_=highA)
                nc.sync.dma_start(
                    out=high_d[c0:c0 + G, 128:256, :].rearrange("c r w -> r c w"), in_=highB)

    nc.compile()
    _NC_CACHE[key] = nc
    return nc


# ----------------------------------------------------------------------------
# host entry points
# ----------------------------------------------------------------------------

_RUNNER = None


def _get_runner():
    """Builds (once) a cached sharded jit executable over the 8 cores.

    Mirrors bass2jax.run_bass_via_pjrt's multi-core path, but without
    donation (the kernel writes every output element, so output buffers
    need not be zero-shipped per call) and with the jitted callable plus
    the device-resident weight/output operands cached across calls.
    """
    global _RUNNER
    if _RUNNER is not None:
        return _RUNNER

    import jax
    from jax.sharding import Mesh, PartitionSpec, NamedSharding
    from jax.experimental.shard_map import shard_map
    import concourse.mybir as mybir
    from concourse import bass2jax
    from concourse.bass2jax import _bass_exec_p, partition_id_tensor

    bass2jax.install_neuronx_cc_hook()
    nc = build_nc(C_)

    partition_name = nc.partition_id_tensor.name if nc.partition_id_tensor else None
    in_names, out_names, out_avals = [], [], []
    for alloc in nc.m.functions[0].allocations:
        if not isinstance(alloc, mybir.MemoryLocationSet):
            continue
        name = alloc.memorylocations[0].name
        if alloc.kind == "ExternalInput":
            if name != partition_name:
                in_names.append(name)
        elif alloc.kind == "ExternalOutput":
            out_names.append(name)
            out_avals.append(jax.core.ShapedArray(
                tuple(alloc.tensor_shape), mybir.dt.np(alloc.dtype)))
    n_params = len(in_names)
    all_in_names = list(in_names) + list(out_names)
    if partition_name is not None:
        all_in_names.append(partition_name)

    def _body(*args):
        operands = list(args)
        if partition_name is not None:
            operands.append(partition_id_tensor())
        return tuple(_bass_exec_p.bind(
            *operands,
            out_avals=tuple(out_avals),
            in_names=tuple(all_in_names),
            out_names=tuple(out_names),
            lowering_input_output_aliases=(),
            sim_require_finite=True,
            sim_require_nnan=True,
            nc=nc,
        ))

    devices = jax.devices()[:NCORES]
    mesh = Mesh(np.asarray(devices), ("core",))
    n_in = n_params + len(out_names)
    sharded = jax.jit(shard_map(
        _body, mesh=mesh,
        in_specs=(PartitionSpec("core"),) * n_in,
        out_specs=(PartitionSpec("core"),) * len(out_names),
        check_rep=False))

    shard0 = NamedSharding(mesh, PartitionSpec("core"))
    wt = _weights()
    # device-resident static operands: weights (replicated per core) and
    # uninitialized-output placeholders
    static = {}
    for name in in_names:
        if name == "x":
            continue
        arr = np.concatenate([wt[name]] * NCORES, axis=0)
        static[name] = jax.device_put(arr, shard0)
    for name, aval in zip(out_names, out_avals):
        z = np.zeros((aval.shape[0] * NCORES,) + tuple(aval.shape[1:]),
                     dtype=aval.dtype)
        static[name] = jax.device_put(z, shard0)

    def run(x_global):
        """x_global: np or jax array [8*64, 256, 256] fp32 (sharded ok)."""
        ops = []
        for name in in_names:
            ops.append(x_global if name == "x" else static[name])
        for name in out_names:
            ops.append(static[name])
        outs = sharded(*ops)
        return dict(zip(out_names, outs))

    _RUNNER = (run, shard0)
    return _RUNNER


def _run_device(x, trace=False):
    """x: [8, 64, 256, 256] fp32. Returns (low, high, results_obj)."""
    if trace:
        from concourse import bass_utils
        nc = build_nc(C_)
        wt = _weights()
        in_maps = [dict(wt, x=np.ascontiguousarray(x[b])) for b in range(NCORES)]
        res = bass_utils.run_bass_kernel_spmd(
            nc, in_maps, core_ids=list(range(NCORES)), trace=True)
        low = np.stack([res.results[b]["low"] for b in range(NCORES)])
        high = np.stack([res.results[b]["high"] for b in range(NCORES)])
        return low, high, res

    run, _ = _get_runner()
    outs = run(np.ascontiguousarray(x).reshape(B_ * C_, H_, W_))
    low = np.asarray(outs["low"]).reshape(B_, C_, H_, W_)
    high = np.asarray(outs["high"]).reshape(B_, C_, H_, W_)
    return low, high, None


def _fallback(x, level):
    """Numpy port of the reference for unexpected shapes/levels."""
    xl = x.astype(np.float64)
    Bb, Cc, H, W = xl.shape
    low = xl
    high = np.zeros_like(xl)

    def up(a, n_r, n_c):
        Mr = _resize_matrix(a.shape[-2], n_r)
        Mc = _resize_matrix(a.shape[-1], n_c)
        return np.einsum("ij,...jk,lk->...il", Mr, a, Mc)

    for lv in range(level):
        stride = 2 ** lv
        if H // stride < 2 or W // stride < 2:
            break
        x00 = low[..., 0:H - 1:stride, 0:W - 1:stride]
        x01 = low[..., 0:H - 1:stride, 1:W:stride]
        x10 = low[..., 1:H:stride, 0:W - 1:stride]
        x11 = low[..., 1:H:stride, 1:W:stride]
        ll = (x00 + x01 + x10 + x11) * 0.25
        lh = (x00 + x01 - x10 - x11) * 0.25
        hl = (x00 - x01 + x10 - x11) * 0.25
        hh = (x00 - x01 - x10 + x11) * 0.25
        ch = np.abs(lh) + np.abs(hl) + np.abs(hh)
        high = high + up(ch, H, W)
        low = up(ll, H, W)
    if level > 0:
        high = high / level
    return low.astype(np.float32), high.astype(np.float32)


def kernel(x, level):
    x = np.asarray(x, dtype=np.float32)
    level = int(level)
    if level != 2 or x.shape != (B_, C_, H_, W_):
        return _fallback(x, level)
    low, high, _ = _run_device(x)
    return low, high



# revision 5
# speedup vs baseline: 1.1854x; 1.1854x over previous
"""HaarWavelet2D (level=2) Trainium2 kernel, v2.

Contract: kernel(x, level) with x [8, 64, 256, 256] fp32, level=2.
Returns (low_freq, high_freq), each [8, 64, 256, 256] fp32 — matching the
jax reference (2-level Haar decomposition with bilinear resizes).

Sharding: data-parallel over batch — core b processes x[b] (64 channels).

v2 layout (validated in model2.py): rows-in-partitions, row-PARITY tiles
everywhere (row = 2*partition + p).  DRAM tensors are declared
[C, 128, 2, 256] so each G-channel group moves with ONE large DMA
(2 KiB descriptors).  Per group:
  s/d   = column-pair sum/diff, both parities in one TT (f32 in, bf16 out)
  ad    = |d| on the scalar engine; row-shifted (sE, adE) via one
          SBUF->SBUF DMA of an interleaved (s,ad) tile
  t1/m/ch0 = row-pair ops, bf16 TT
  vertical resizes = TensorE matmuls (parity-split banded matrices) in
          Gp=2 chunks, PSUM drained to bf16 SBUF by the scalar engine
  horizontal 255->256 = direct misaligned-bf16 TT diff + W0 mult + add
  level-1 repeats at half size; 128->256 horizontal via diffL tile and
  scalar_tensor_tensor writes straight into the f32 output tiles
Engine assignment (vector/scalar/gpsimd) per op class is parameterized
for rebalancing.
"""

import sys

if "/opt/trn_rl_repo" not in sys.path:
    sys.path.insert(0, "/opt/trn_rl_repo")

import numpy as np
import ml_dtypes

BF = ml_dtypes.bfloat16

B_, C_, H_, W_ = 8, 64, 256, 256
NCORES = 8
G = 8            # channels per group
GP0 = 2          # channels per level-0 matmul chunk
GP1 = 4          # channels per level-1 matmul chunk


# ----------------------------------------------------------------------------
# host-side weight construction
# ----------------------------------------------------------------------------

def _resize_matrix(n, N):
    M = np.zeros((N, n), dtype=np.float64)
    for i in range(N):
        c = (i + 0.5) * n / N - 0.5
        j0 = int(np.floor(c))
        f = c - j0
        M[i, min(max(j0, 0), n - 1)] += 1.0 - f
        M[i, min(max(j0 + 1, 0), n - 1)] += f
    return M


def _build_weights():
    V255 = _resize_matrix(255, 256)
    V128 = _resize_matrix(128, 256)
    Sv1 = np.zeros((255, 256))
    for r in range(255):
        Sv1[r, r] = 1.0
        Sv1[r, r + 1] = 1.0
    Va = 0.25 * (V255 @ Sv1)      # [256, 256]
    Vh = 0.25 * V255              # [256, 255]
    Vq = 0.25 * V128              # [256, 128]
    W0 = np.array([V255[i, i - 1] if i >= 1 else 0.0 for i in range(256)])

    w = {}
    for p in range(2):
        w[f"w_va_{p}e"] = Va[p::2, 0::2].T        # [128,128]
        w[f"w_va_{p}o"] = Va[p::2, 1::2].T        # [128,128]
        w[f"w_vh_{p}e"] = Vh[p::2, 0::2].T        # [128,128]
        w[f"w_vh_{p}o"] = Vh[p::2, 1::2].T        # [127,128]
        w[f"w_vq_{p}"] = Vq[p::2, :].T            # [128,128]
    w["w0t"] = np.tile(W0[None, :], (128, 1))     # [128,256]
    return {k: v.astype(BF) for k, v in w.items()}


_WEIGHTS = None


def _weights():
    global _WEIGHTS
    if _WEIGHTS is None:
        _WEIGHTS = _build_weights()
    return _WEIGHTS


# ----------------------------------------------------------------------------
# bass program
# ----------------------------------------------------------------------------

_NC_CACHE = {}


def build_nc(C=C_):
    key = C
    if key in _NC_CACHE:
        return _NC_CACHE[key]

    import concourse.bass as bass
    import concourse.bacc as bacc
    import concourse.tile as tile
    import concourse.mybir as mybir

    F32 = mybir.dt.float32
    BF16 = mybir.dt.bfloat16
    Alu = mybir.AluOpType
    Act = mybir.ActivationFunctionType
    P = 128

    nc = bacc.Bacc("TRN2", target_bir_lowering=False)
    # [C, 256, 256] viewed as [C, 128, 2, 256]: row = 2r + p
    x_d = nc.dram_tensor("x", [C, P, 2, W_], F32, kind="ExternalInput")
    wt = _weights()
    w_d = {
        name: nc.dram_tensor(name, list(arr.shape), BF16, kind="ExternalInput")
        for name, arr in wt.items()
    }
    low_d = nc.dram_tensor("low", [C, P, 2, W_], F32, kind="ExternalOutput")
    high_d = nc.dram_tensor("high", [C, P, 2, W_], F32, kind="ExternalOutput")

    # engine assignment knobs
    ENG_DIFF255 = "gpsimd"   # rh255 diff TT
    ENG_S2D2 = "gpsimd"      # level-1 column pair ops
    ENG_DIFF128 = "gpsimd"   # rh128 diffL TT
    ENG_DRAIN0 = "scalar"    # level-0 PSUM drains
    ENG_DRAIN1 = "scalar"    # level-1 PSUM drains

    with tile.TileContext(nc) as tc:
        with (
            tc.tile_pool(name="consts", bufs=1) as consts,
            tc.tile_pool(name="xin", bufs=1) as xin,
            tc.tile_pool(name="front", bufs=1) as front,
            tc.tile_pool(name="qp", bufs=1) as qp,
            tc.tile_pool(name="hor", bufs=1) as hor,
            tc.tile_pool(name="lv1", bufs=1) as lv1,
            tc.tile_pool(name="outp", bufs=1) as outp,
            tc.tile_pool(name="ps0", bufs=1, space="PSUM") as ps0,
        ):
            eng = {
                "vector": nc.vector,
                "scalar": nc.scalar,
                "gpsimd": nc.gpsimd,
            }

            def copy_op(engine_name, out, in_):
                e = eng[engine_name]
                if engine_name == "scalar":
                    e.copy(out=out, in_=in_)
                else:
                    e.tensor_copy(out=out, in_=in_)

            wtile = {}
            for name, arr in wt.items():
                t = consts.tile(list(arr.shape), BF16, tag=name)
                nc.sync.dma_start(out=t, in_=w_d[name][:, :])
                wtile[name] = t

            # persistent tiles (bufs=1 pools -> stable addresses)
            qL = qp.tile([P, 2, G, 256], BF16, tag="qL")
            qh = qp.tile([P, 2, G, 256], BF16, tag="qh")
            diffT_L = hor.tile([P, 2, G, 256], BF16, tag="diffT_L")
            diffT_h = hor.tile([P, 2, G, 256], BF16, tag="diffT_h")
            qlo = lv1.tile([P, 2, G, 128], BF16, tag="qlo")
            qh1 = lv1.tile([P, 2, G, 128], BF16, tag="qh1")
            diffL_lo = lv1.tile([P, 2, G, 129], BF16, tag="diffL_lo")
            diffL_h1 = lv1.tile([P, 2, G, 129], BF16, tag="diffL_h1")

            # one-time edge memsets (values cancel algebraically; avoids NaNs)
            nc.vector.memset(qL[:, :, :, 255:256], 0.0)
            nc.vector.memset(qh[:, :, :, 255:256], 0.0)
            nc.vector.memset(diffT_L[:, :, :, 0:1], 0.0)
            nc.vector.memset(diffT_h[:, :, :, 0:1], 0.0)
            for dl in (diffL_lo, diffL_h1):
                nc.vector.memset(dl[:, :, :, 0:1], 0.0)
                nc.vector.memset(dl[:, :, :, 128:129], 0.0)

            w0b = wtile["w0t"][:, :]
            w0_ap = bass.AP(tensor=w0b.tensor, offset=w0b.offset,
                            ap=[w0b.ap[0], [0, 2], [0, G], w0b.ap[1]])

            n_iter = C // G
            for it in range(n_iter):
                c0 = it * G

                # ---- load -------------------------------------------------
                X = xin.tile([P, G, 2, 256], F32, tag="X")
                nc.sync.dma_start(
                    out=X, in_=x_d[c0:c0 + G].rearrange("c r p w -> r c p w"))

                # ---- level-0 column pairs ---------------------------------
                SDA = front.tile([P, G, 4, 255], BF16, tag="SDA")
                d0 = front.tile([P, G, 2, 255], BF16, tag="d0")
                nc.vector.tensor_tensor(
                    out=SDA[:, :, 0:2, :], in0=X[:, :, :, 0:255],
                    in1=X[:, :, :, 1:256], op=Alu.add)
                nc.vector.tensor_tensor(
                    out=d0, in0=X[:, :, :, 0:255],
                    in1=X[:, :, :, 1:256], op=Alu.subtract)
                nc.scalar.activation(out=SDA[:, :, 2:4, :], in_=d0, func=Act.Abs)

                SDA2 = front.tile([127, G, 2, 255], BF16, tag="SDA2")
                nc.sync.dma_start(out=SDA2, in_=SDA[1:128, :, 0:4:2, :])

                # ---- level-0 row pairs ------------------------------------
                t1 = front.tile([P, G, 2, 255], BF16, tag="t1")
                m0 = front.tile([P, G, 2, 255], BF16, tag="m0")
                a1 = front.tile([P, G, 2, 255], BF16, tag="a1")
                ch0 = front.tile([P, G, 2, 255], BF16, tag="ch0")
                nc.vector.tensor_tensor(
                    out=t1[:, :, 0, :], in0=SDA[:, :, 0, :], in1=SDA[:, :, 1, :],
                    op=Alu.subtract)
                nc.vector.tensor_tensor(
                    out=t1[0:127, :, 1, :], in0=SDA[0:127, :, 1, :],
                    in1=SDA2[:, :, 0, :], op=Alu.subtract)
                nc.vector.tensor_tensor(
                    out=m0[:, :, 0, :], in0=SDA[:, :, 2, :], in1=SDA[:, :, 3, :],
                    op=Alu.max)
                nc.vector.tensor_tensor(
                    out=m0[0:127, :, 1, :], in0=SDA[0:127, :, 3, :],
                    in1=SDA2[:, :, 1, :], op=Alu.max)
                nc.scalar.activation(
                    out=a1[:, :, 0, :], in_=t1[:, :, 0, :], func=Act.Abs, scale=0.5)
                nc.scalar.activation(
                    out=a1[0:127, :, 1, :], in_=t1[0:127, :, 1, :], func=Act.Abs,
                    scale=0.5)
                nc.vector.tensor_tensor(
                    out=ch0[:, :, 0, :], in0=a1[:, :, 0, :], in1=m0[:, :, 0, :],
                    op=Alu.add)
                nc.vector.tensor_tensor(
                    out=ch0[0:127, :, 1, :], in0=a1[0:127, :, 1, :],
                    in1=m0[0:127, :, 1, :], op=Alu.add)

                # ---- level-0 vertical matmuls + drains --------------------
                NF0 = GP0 * 255
                for cc in range(0, G, GP0):
                    Y_L = ps0.tile([P, 2, 512], F32, tag="Y_L")
                    Y_h = ps0.tile([P, 2, 512], F32, tag="Y_h")
                    for p in range(2):
                        nc.tensor.matmul(
                            out=Y_L[:, p, 0:NF0], lhsT=wtile[f"w_va_{p}e"][:, :],
                            rhs=SDA[:, cc:cc + GP0, 0, :], start=True, stop=False)
                        nc.tensor.matmul(
                            out=Y_L[:, p, 0:NF0], lhsT=wtile[f"w_va_{p}o"][:, :],
                            rhs=SDA[:, cc:cc + GP0, 1, :], start=False, stop=True)
                        nc.tensor.matmul(
                            out=Y_h[:, p, 0:NF0], lhsT=wtile[f"w_vh_{p}e"][:, :],
                            rhs=ch0[:, cc:cc + GP0, 0, :], start=True, stop=False)
                        nc.tensor.matmul(
                            out=Y_h[:, p, 0:NF0], lhsT=wtile[f"w_vh_{p}o"][:, :],
                            rhs=ch0[0:127, cc:cc + GP0, 1, :], start=False,
                            stop=True)
                    copy_op(
                        ENG_DRAIN0, qL[:, :, cc:cc + GP0, 0:255],
                        Y_L[:, :, 0:NF0].rearrange("r p (g w) -> r p g w", w=255))
                    copy_op(
                        ENG_DRAIN0, qh[:, :, cc:cc + GP0, 0:255],
                        Y_h[:, :, 0:NF0].rearrange("r p (g w) -> r p g w", w=255))

                # ---- level-0 horizontal resize (255->256) -----------------
                L0x = hor.tile([P, 2, G, 256], BF16, tag="L0x")
                h0x = hor.tile([P, 2, G, 256], BF16, tag="h0x")
                tmpT = hor.tile([P, 2, G, 256], BF16, tag="tmpT")
                for q, diffT, t, out in (
                    (qL, diffT_L, tmpT, L0x),
                    (qh, diffT_h, tmpT, h0x),
                ):
                    eng[ENG_DIFF255].tensor_tensor(
                        out=diffT[:, :, :, 1:256], in0=q[:, :, :, 0:255],
                        in1=q[:, :, :, 1:256], op=Alu.subtract)
                    nc.vector.tensor_tensor(
                        out=t, in0=diffT, in1=w0_ap, op=Alu.mult)
                    nc.vector.tensor_tensor(out=out, in0=q, in1=t, op=Alu.add)

                # ---- level-1 ----------------------------------------------
                s2 = lv1.tile([P, 2, G, 128], BF16, tag="s2")
                d2 = lv1.tile([P, 2, G, 128], BF16, tag="d2")
                ad2 = lv1.tile([P, 2, G, 128], BF16, tag="ad2")
                eng[ENG_S2D2].tensor_tensor(
                    out=s2, in0=L0x[:, :, :, 0:256:2], in1=L0x[:, :, :, 1:256:2],
                    op=Alu.add)
                eng[ENG_S2D2].tensor_tensor(
                    out=d2, in0=L0x[:, :, :, 0:256:2], in1=L0x[:, :, :, 1:256:2],
                    op=Alu.subtract)
                nc.scalar.activation(out=ad2, in_=d2, func=Act.Abs)
                lsum1 = lv1.tile([P, G, 128], BF16, tag="lsum1")
                t1b = lv1.tile([P, G, 128], BF16, tag="t1b")
                m1 = lv1.tile([P, G, 128], BF16, tag="m1")
                a1b = lv1.tile([P, G, 128], BF16, tag="a1b")
                ch1 = lv1.tile([P, G, 128], BF16, tag="ch1")
                nc.vector.tensor_tensor(
                    out=lsum1, in0=s2[:, 0], in1=s2[:, 1], op=Alu.add)
                nc.vector.tensor_tensor(
                    out=t1b, in0=s2[:, 0], in1=s2[:, 1], op=Alu.subtract)
                nc.vector.tensor_tensor(
                    out=m1, in0=ad2[:, 0], in1=ad2[:, 1], op=Alu.max)
                nc.scalar.activation(out=a1b, in_=t1b, func=Act.Abs, scale=0.5)
                nc.vector.tensor_tensor(
                    out=ch1, in0=a1b, in1=m1, op=Alu.add)

                # ---- level-1 vertical matmuls + drains --------------------
                NF1 = GP1 * 128
                for cc in range(0, G, GP1):
                    Y_lo = ps0.tile([P, 2, 512], F32, tag="Y_lo")
                    Y_h1 = ps0.tile([P, 2, 512], F32, tag="Y_h1")
                    for p in range(2):
                        nc.tensor.matmul(
                            out=Y_lo[:, p, 0:NF1], lhsT=wtile[f"w_vq_{p}"][:, :],
                            rhs=lsum1[:, cc:cc + GP1, :], start=True, stop=True)
                        nc.tensor.matmul(
                            out=Y_h1[:, p, 0:NF1], lhsT=wtile[f"w_vq_{p}"][:, :],
                            rhs=ch1[:, cc:cc + GP1, :], start=True, stop=True)
                    copy_op(
                        ENG_DRAIN1, qlo[:, :, cc:cc + GP1, :],
                        Y_lo[:, :, 0:NF1].rearrange("r p (g w) -> r p g w", w=128))
                    copy_op(
                        ENG_DRAIN1, qh1[:, :, cc:cc + GP1, :],
                        Y_h1[:, :, 0:NF1].rearrange("r p (g w) -> r p g w", w=128))

                # ---- level-1 horizontal diffs -----------------------------
                for q, dl in ((qlo, diffL_lo), (qh1, diffL_h1)):
                    eng[ENG_DIFF128].tensor_tensor(
                        out=dl[:, :, :, 1:128], in0=q[:, :, :, 0:127],
                        in1=q[:, :, :, 1:128], op=Alu.subtract)

                # ---- finalize ---------------------------------------------
                lowT = outp.tile([P, G, 2, 256], BF16, tag="lowT")
                highT = outp.tile([P, G, 2, 256], BF16, tag="highT")
                h1x = lv1.tile([P, 2, 2, G, 128], BF16, tag="h1x")
                for p in range(2):
                    nc.vector.scalar_tensor_tensor(
                        out=lowT[:, :, p, 0:256:2], in0=diffL_lo[:, p, :, 0:128],
                        scalar=0.25, in1=qlo[:, p], op0=Alu.mult, op1=Alu.add)
                    nc.vector.scalar_tensor_tensor(
                        out=lowT[:, :, p, 1:256:2], in0=diffL_lo[:, p, :, 1:129],
                        scalar=-0.25, in1=qlo[:, p], op0=Alu.mult, op1=Alu.add)
                    nc.vector.scalar_tensor_tensor(
                        out=h1x[:, p, 0], in0=diffL_h1[:, p, :, 0:128],
                        scalar=0.25, in1=qh1[:, p], op0=Alu.mult, op1=Alu.add)
                    nc.vector.scalar_tensor_tensor(
                        out=h1x[:, p, 1], in0=diffL_h1[:, p, :, 1:129],
                        scalar=-0.25, in1=qh1[:, p], op0=Alu.mult, op1=Alu.add)
                    nc.vector.tensor_tensor(
                        out=highT[:, :, p, 0:256:2], in0=h1x[:, p, 0],
                        in1=h0x[:, p, :, 0:256:2], op=Alu.add)
                    nc.vector.tensor_tensor(
                        out=highT[:, :, p, 1:256:2], in0=h1x[:, p, 1],
                        in1=h0x[:, p, :, 1:256:2], op=Alu.add)

                # ---- store ------------------------------------------------
                nc.gpsimd.dma_start(
                    out=low_d[c0:c0 + G].rearrange("c r p w -> r c p w"),
                    in_=lowT)
                nc.gpsimd.dma_start(
                    out=high_d[c0:c0 + G].rearrange("c r p w -> r c p w"),
                    in_=highT)

    nc.compile()
    _NC_CACHE[key] = nc
    return nc


# ----------------------------------------------------------------------------
# host entry points
# ----------------------------------------------------------------------------

_RUNNER = None


def _get_runner():
    """Builds (once) a cached sharded jit executable over the 8 cores."""
    global _RUNNER
    if _RUNNER is not None:
        return _RUNNER

    import jax
    from jax.sharding import Mesh, PartitionSpec, NamedSharding
    from jax.experimental.shard_map import shard_map
    import concourse.mybir as mybir
    from concourse import bass2jax
    from concourse.bass2jax import _bass_exec_p, partition_id_tensor

    bass2jax.install_neuronx_cc_hook()
    nc = build_nc(C_)

    partition_name = nc.partition_id_tensor.name if nc.partition_id_tensor else None
    in_names, out_names, out_avals = [], [], []
    for alloc in nc.m.functions[0].allocations:
        if not isinstance(alloc, mybir.MemoryLocationSet):
            continue
        name = alloc.memorylocations[0].name
        if alloc.kind == "ExternalInput":
            if name != partition_name:
                in_names.append(name)
        elif alloc.kind == "ExternalOutput":
            out_names.append(name)
            out_avals.append(jax.core.ShapedArray(
                tuple(alloc.tensor_shape), mybir.dt.np(alloc.dtype)))
    n_params = len(in_names)
    all_in_names = list(in_names) + list(out_names)
    if partition_name is not None:
        all_in_names.append(partition_name)

    def _body(*args):
        operands = list(args)
        if partition_name is not None:
            operands.append(partition_id_tensor())
        return tuple(_bass_exec_p.bind(
            *operands,
            out_avals=tuple(out_avals),
            in_names=tuple(all_in_names),
            out_names=tuple(out_names),
            lowering_input_output_aliases=(),
            sim_require_finite=True,
            sim_require_nnan=True,
            nc=nc,
        ))

    devices = jax.devices()[:NCORES]
    mesh = Mesh(np.asarray(devices), ("core",))
    n_in = n_params + len(out_names)
    sharded = jax.jit(shard_map(
        _body, mesh=mesh,
        in_specs=(PartitionSpec("core"),) * n_in,
        out_specs=(PartitionSpec("core"),) * len(out_names),
        check_rep=False))

    shard0 = NamedSharding(mesh, PartitionSpec("core"))
    wt = _weights()
    static = {}
    for name in in_names:
        if name == "x":
            continue
        arr = np.concatenate([wt[name]] * NCORES, axis=0)
        static[name] = jax.device_put(arr, shard0)
    for name, aval in zip(out_names, out_avals):
        z = np.zeros((aval.shape[0] * NCORES,) + tuple(aval.shape[1:]),
                     dtype=aval.dtype)
        static[name] = jax.device_put(z, shard0)

    def run(x_global):
        ops = []
        for name in in_names:
            ops.append(x_global if name == "x" else static[name])
        for name in out_names:
            ops.append(static[name])
        outs = sharded(*ops)
        return dict(zip(out_names, outs))

    _RUNNER = (run, shard0)
    return _RUNNER


def _run_device(x, trace=False):
    """x: [8, 64, 256, 256] fp32. Returns (low, high, results_obj)."""
    if trace:
        from concourse import bass_utils
        nc = build_nc(C_)
        wt = _weights()
        in_maps = [
            dict(wt, x=np.ascontiguousarray(x[b]).reshape(C_, 128, 2, W_))
            for b in range(NCORES)
        ]
        res = bass_utils.run_bass_kernel_spmd(
            nc, in_maps, core_ids=list(range(NCORES)), trace=True)
        low = np.stack([
            res.results[b]["low"].reshape(C_, H_, W_) for b in range(NCORES)])
        high = np.stack([
            res.results[b]["high"].reshape(C_, H_, W_) for b in range(NCORES)])
        return low, high, res

    run, _ = _get_runner()
    outs = run(np.ascontiguousarray(x).reshape(B_ * C_, 128, 2, W_))
    low = np.asarray(outs["low"]).reshape(B_, C_, H_, W_)
    high = np.asarray(outs["high"]).reshape(B_, C_, H_, W_)
    return low, high, None


def _fallback(x, level):
    """Numpy port of the reference for unexpected shapes/levels."""
    xl = x.astype(np.float64)
    low = xl
    high = np.zeros_like(xl)
    Bb, Cc, H, W = xl.shape

    def up(a, n_r, n_c):
        Mr = _resize_matrix(a.shape[-2], n_r)
        Mc = _resize_matrix(a.shape[-1], n_c)
        return np.einsum("ij,...jk,lk->...il", Mr, a, Mc)

    for lv in range(level):
        stride = 2 ** lv
        if H // stride < 2 or W // stride < 2:
            break
        x00 = low[..., 0:H - 1:stride, 0:W - 1:stride]
        x01 = low[..., 0:H - 1:stride, 1:W:stride]
        x10 = low[..., 1:H:stride, 0:W - 1:stride]
        x11 = low[..., 1:H:stride, 1:W:stride]
        ll = (x00 + x01 + x10 + x11) * 0.25
        lh = (x00 + x01 - x10 - x11) * 0.25
        hl = (x00 - x01 + x10 - x11) * 0.25
        hh = (x00 - x01 - x10 + x11) * 0.25
        ch = np.abs(lh) + np.abs(hl) + np.abs(hh)
        high = high + up(ch, H, W)
        low = up(ll, H, W)
    if level > 0:
        high = high / level
    return low.astype(np.float32), high.astype(np.float32)


def kernel(x, level):
    x = np.asarray(x, dtype=np.float32)
    level = int(level)
    if level != 2 or x.shape != (B_, C_, H_, W_):
        return _fallback(x, level)
    low, high, _ = _run_device(x)
    return low, high


# revision 6
# speedup vs baseline: 1.1983x; 1.0108x over previous
"""HaarWavelet2D (level=2) Trainium2 kernel, v2.

Contract: kernel(x, level) with x [8, 64, 256, 256] fp32, level=2.
Returns (low_freq, high_freq), each [8, 64, 256, 256] fp32 — matching the
jax reference (2-level Haar decomposition with bilinear resizes).

Sharding: data-parallel over batch — core b processes x[b] (64 channels).

v2 layout (validated in model2.py): rows-in-partitions, row-PARITY tiles
everywhere (row = 2*partition + p).  DRAM tensors are declared
[C, 128, 2, 256] so each G-channel group moves with ONE large DMA
(2 KiB descriptors).  Per group:
  s/d   = column-pair sum/diff, both parities in one TT (f32 in, bf16 out)
  ad    = |d| on the scalar engine; row-shifted (sE, adE) via one
          SBUF->SBUF DMA of an interleaved (s,ad) tile
  t1/m/ch0 = row-pair ops, bf16 TT
  vertical resizes = TensorE matmuls (parity-split banded matrices) in
          Gp=2 chunks, PSUM drained to bf16 SBUF by the scalar engine
  horizontal 255->256 = direct misaligned-bf16 TT diff + W0 mult + add
  level-1 repeats at half size; 128->256 horizontal via diffL tile and
  scalar_tensor_tensor writes straight into the f32 output tiles
Engine assignment (vector/scalar/gpsimd) per op class is parameterized
for rebalancing.
"""

import sys

if "/opt/trn_rl_repo" not in sys.path:
    sys.path.insert(0, "/opt/trn_rl_repo")

import numpy as np
import ml_dtypes

BF = ml_dtypes.bfloat16

B_, C_, H_, W_ = 8, 64, 256, 256
NCORES = 8
G = 4            # channels per group
GP0 = 2          # channels per level-0 matmul chunk
GP1 = 4          # channels per level-1 matmul chunk


# ----------------------------------------------------------------------------
# host-side weight construction
# ----------------------------------------------------------------------------

def _resize_matrix(n, N):
    M = np.zeros((N, n), dtype=np.float64)
    for i in range(N):
        c = (i + 0.5) * n / N - 0.5
        j0 = int(np.floor(c))
        f = c - j0
        M[i, min(max(j0, 0), n - 1)] += 1.0 - f
        M[i, min(max(j0 + 1, 0), n - 1)] += f
    return M


def _build_weights():
    V255 = _resize_matrix(255, 256)
    V128 = _resize_matrix(128, 256)
    Sv1 = np.zeros((255, 256))
    for r in range(255):
        Sv1[r, r] = 1.0
        Sv1[r, r + 1] = 1.0
    Va = 0.25 * (V255 @ Sv1)      # [256, 256]
    Vh = 0.25 * V255              # [256, 255]
    Vq = 0.25 * V128              # [256, 128]
    W0 = np.array([V255[i, i - 1] if i >= 1 else 0.0 for i in range(256)])

    w = {}
    for p in range(2):
        w[f"w_va_{p}e"] = Va[p::2, 0::2].T        # [128,128]
        w[f"w_va_{p}o"] = Va[p::2, 1::2].T        # [128,128]
        w[f"w_vh_{p}e"] = Vh[p::2, 0::2].T        # [128,128]
        w[f"w_vh_{p}o"] = Vh[p::2, 1::2].T        # [127,128]
        w[f"w_vq_{p}"] = Vq[p::2, :].T            # [128,128]
    w["w0t"] = np.tile(W0[None, :], (128, 1))     # [128,256]
    return {k: v.astype(BF) for k, v in w.items()}


_WEIGHTS = None


def _weights():
    global _WEIGHTS
    if _WEIGHTS is None:
        _WEIGHTS = _build_weights()
    return _WEIGHTS


# ----------------------------------------------------------------------------
# bass program
# ----------------------------------------------------------------------------

_NC_CACHE = {}


def build_nc(C=C_):
    key = C
    if key in _NC_CACHE:
        return _NC_CACHE[key]

    import concourse.bass as bass
    import concourse.bacc as bacc
    import concourse.tile as tile
    import concourse.mybir as mybir

    F32 = mybir.dt.float32
    BF16 = mybir.dt.bfloat16
    Alu = mybir.AluOpType
    Act = mybir.ActivationFunctionType
    P = 128

    nc = bacc.Bacc("TRN2", target_bir_lowering=False)
    # [C, 256, 256] viewed as [C, 128, 2, 256]: row = 2r + p
    x_d = nc.dram_tensor("x", [C, P, 2, W_], F32, kind="ExternalInput")
    wt = _weights()
    w_d = {
        name: nc.dram_tensor(name, list(arr.shape), BF16, kind="ExternalInput")
        for name, arr in wt.items()
    }
    low_d = nc.dram_tensor("low", [C, P, 2, W_], F32, kind="ExternalOutput")
    high_d = nc.dram_tensor("high", [C, P, 2, W_], F32, kind="ExternalOutput")

    with tile.TileContext(nc) as tc:
        with (
            tc.tile_pool(name="consts", bufs=1) as consts,
            tc.tile_pool(name="xin", bufs=2) as xin,
            tc.tile_pool(name="front", bufs=2) as front,
            tc.tile_pool(name="qp", bufs=1) as qp,
            tc.tile_pool(name="hor", bufs=2) as hor,
            tc.tile_pool(name="lv1", bufs=2) as lv1,
            tc.tile_pool(name="outp", bufs=2) as outp,
            tc.tile_pool(name="ps0", bufs=1, space="PSUM") as ps0,
        ):
            wtile = {}
            for name, arr in wt.items():
                t = consts.tile(list(arr.shape), BF16, tag=name)
                nc.sync.dma_start(out=t, in_=w_d[name][:, :])
                wtile[name] = t

            # persistent tiles (bufs=1 qp pool -> stable addresses for the
            # one-time edge memsets; values cancel algebraically, the memset
            # only guards against NaN garbage)
            qL = qp.tile([P, 2, G, 256], BF16, tag="qL")
            qh = qp.tile([P, 2, G, 256], BF16, tag="qh")
            diffT_L = qp.tile([P, 2, G, 256], BF16, tag="diffT_L")
            diffT_h = qp.tile([P, 2, G, 256], BF16, tag="diffT_h")
            qlo = qp.tile([P, 2, G, 128], BF16, tag="qlo")
            qh1 = qp.tile([P, 2, G, 128], BF16, tag="qh1")
            diffL_lo = qp.tile([P, 2, G, 129], BF16, tag="diffL_lo")
            diffL_h1 = qp.tile([P, 2, G, 129], BF16, tag="diffL_h1")

            nc.vector.memset(qL[:, :, :, 255:256], 0.0)
            nc.vector.memset(qh[:, :, :, 255:256], 0.0)
            nc.vector.memset(diffT_L[:, :, :, 0:1], 0.0)
            nc.vector.memset(diffT_h[:, :, :, 0:1], 0.0)
            for dl in (diffL_lo, diffL_h1):
                nc.vector.memset(dl[:, :, :, 0:1], 0.0)
                nc.vector.memset(dl[:, :, :, 128:129], 0.0)

            w0b = wtile["w0t"][:, :]
            w0_ap = bass.AP(tensor=w0b.tensor, offset=w0b.offset,
                            ap=[w0b.ap[0], [0, 2], [0, G], w0b.ap[1]])

            n_iter = C // G
            for it in range(n_iter):
                c0 = it * G

                # ---- load -------------------------------------------------
                X = xin.tile([P, G, 2, 256], F32, tag="X")
                nc.sync.dma_start(
                    out=X, in_=x_d[c0:c0 + G].rearrange("c r p w -> r c p w"))

                # ---- level-0 column pairs (parity-first outputs) ----------
                SDA = front.tile([P, 4, G, 255], BF16, tag="SDA")
                d0 = front.tile([P, 2, G, 255], BF16, tag="d0")
                Xp = X[:, :, :, 0:255].rearrange("r g p w -> r p g w")
                Xp1 = X[:, :, :, 1:256].rearrange("r g p w -> r p g w")
                nc.vector.tensor_tensor(
                    out=SDA[:, 0:2], in0=Xp, in1=Xp1, op=Alu.add)
                nc.vector.tensor_tensor(
                    out=d0, in0=Xp, in1=Xp1, op=Alu.subtract)
                nc.scalar.activation(out=SDA[:, 2:4], in_=d0, func=Act.Abs)

                SDA2 = front.tile([127, 2, G, 255], BF16, tag="SDA2")
                nc.sync.dma_start(out=SDA2, in_=SDA[1:128, 0:4:2])

                # ---- level-0 row pairs ------------------------------------
                t1 = front.tile([P, 2, G, 255], BF16, tag="t1")
                m0 = front.tile([P, 2, G, 255], BF16, tag="m0")
                a1 = front.tile([P, 2, G, 255], BF16, tag="a1")
                ch0 = front.tile([P, 2, G, 255], BF16, tag="ch0")
                nc.vector.tensor_tensor(
                    out=t1[:, 0], in0=SDA[:, 0], in1=SDA[:, 1], op=Alu.subtract)
                nc.vector.tensor_tensor(
                    out=t1[0:127, 1], in0=SDA[0:127, 1], in1=SDA2[:, 0],
                    op=Alu.subtract)
                nc.vector.tensor_tensor(
                    out=m0[:, 0], in0=SDA[:, 2], in1=SDA[:, 3], op=Alu.max)
                nc.vector.tensor_tensor(
                    out=m0[0:127, 1], in0=SDA[0:127, 3], in1=SDA2[:, 1],
                    op=Alu.max)
                nc.scalar.activation(
                    out=a1[:, 0], in_=t1[:, 0], func=Act.Abs, scale=0.5)
                nc.scalar.activation(
                    out=a1[0:127, 1], in_=t1[0:127, 1], func=Act.Abs, scale=0.5)
                nc.vector.tensor_tensor(
                    out=ch0[:, 0], in0=a1[:, 0], in1=m0[:, 0], op=Alu.add)
                nc.vector.tensor_tensor(
                    out=ch0[0:127, 1], in0=a1[0:127, 1], in1=m0[0:127, 1],
                    op=Alu.add)

                # ---- level-0 vertical matmuls + drains --------------------
                NF0 = GP0 * 255
                for cc in range(0, G, GP0):
                    Y_L = ps0.tile([P, 2, 512], F32, tag="Y_L")
                    Y_h = ps0.tile([P, 2, 512], F32, tag="Y_h")
                    for p in range(2):
                        nc.tensor.matmul(
                            out=Y_L[:, p, 0:NF0], lhsT=wtile[f"w_va_{p}e"][:, :],
                            rhs=SDA[:, 0, cc:cc + GP0, :], start=True, stop=False)
                        nc.tensor.matmul(
                            out=Y_L[:, p, 0:NF0], lhsT=wtile[f"w_va_{p}o"][:, :],
                            rhs=SDA[:, 1, cc:cc + GP0, :], start=False, stop=True)
                        nc.tensor.matmul(
                            out=Y_h[:, p, 0:NF0], lhsT=wtile[f"w_vh_{p}e"][:, :],
                            rhs=ch0[:, 0, cc:cc + GP0, :], start=True, stop=False)
                        nc.tensor.matmul(
                            out=Y_h[:, p, 0:NF0], lhsT=wtile[f"w_vh_{p}o"][:, :],
                            rhs=ch0[0:127, 1, cc:cc + GP0, :], start=False,
                            stop=True)
                    nc.scalar.copy(
                        out=qL[:, :, cc:cc + GP0, 0:255],
                        in_=Y_L[:, :, 0:NF0].rearrange("r p (g w) -> r p g w", w=255))
                    nc.scalar.copy(
                        out=qh[:, :, cc:cc + GP0, 0:255],
                        in_=Y_h[:, :, 0:NF0].rearrange("r p (g w) -> r p g w", w=255))

                # ---- level-0 horizontal resize (255->256) -----------------
                L0x = hor.tile([P, 2, G, 256], BF16, tag="L0x")
                h0x = hor.tile([P, 2, G, 256], BF16, tag="h0x")
                tmpT = hor.tile([P, 2, G, 256], BF16, tag="tmpT")
                for q, diffT, out in (
                    (qL, diffT_L, L0x),
                    (qh, diffT_h, h0x),
                ):
                    nc.vector.tensor_tensor(
                        out=diffT[:, :, :, 1:256], in0=q[:, :, :, 0:255],
                        in1=q[:, :, :, 1:256], op=Alu.subtract)
                    nc.vector.tensor_tensor(
                        out=tmpT, in0=diffT, in1=w0_ap, op=Alu.mult)
                    nc.vector.tensor_tensor(out=out, in0=q, in1=tmpT, op=Alu.add)

                # ---- level-1 ----------------------------------------------
                s2 = lv1.tile([P, 2, G, 128], BF16, tag="s2")
                d2 = lv1.tile([P, 2, G, 128], BF16, tag="d2")
                ad2 = lv1.tile([P, 2, G, 128], BF16, tag="ad2")
                nc.vector.tensor_tensor(
                    out=s2, in0=L0x[:, :, :, 0:256:2], in1=L0x[:, :, :, 1:256:2],
                    op=Alu.add)
                nc.vector.tensor_tensor(
                    out=d2, in0=L0x[:, :, :, 0:256:2], in1=L0x[:, :, :, 1:256:2],
                    op=Alu.subtract)
                nc.scalar.activation(out=ad2, in_=d2, func=Act.Abs)
                lsum1 = lv1.tile([P, G, 128], BF16, tag="lsum1")
                t1b = lv1.tile([P, G, 128], BF16, tag="t1b")
                m1 = lv1.tile([P, G, 128], BF16, tag="m1")
                a1b = lv1.tile([P, G, 128], BF16, tag="a1b")
                ch1 = lv1.tile([P, G, 128], BF16, tag="ch1")
                nc.vector.tensor_tensor(
                    out=lsum1, in0=s2[:, 0], in1=s2[:, 1], op=Alu.add)
                nc.vector.tensor_tensor(
                    out=t1b, in0=s2[:, 0], in1=s2[:, 1], op=Alu.subtract)
                nc.vector.tensor_tensor(
                    out=m1, in0=ad2[:, 0], in1=ad2[:, 1], op=Alu.max)
                nc.scalar.activation(out=a1b, in_=t1b, func=Act.Abs, scale=0.5)
                nc.vector.tensor_tensor(
                    out=ch1, in0=a1b, in1=m1, op=Alu.add)

                # ---- level-1 vertical matmuls + drains --------------------
                NF1 = GP1 * 128
                for cc in range(0, G, GP1):
                    Y_lo = ps0.tile([P, 2, 512], F32, tag="Y_lo")
                    Y_h1 = ps0.tile([P, 2, 512], F32, tag="Y_h1")
                    for p in range(2):
                        nc.tensor.matmul(
                            out=Y_lo[:, p, 0:NF1], lhsT=wtile[f"w_vq_{p}"][:, :],
                            rhs=lsum1[:, cc:cc + GP1, :], start=True, stop=True)
                        nc.tensor.matmul(
                            out=Y_h1[:, p, 0:NF1], lhsT=wtile[f"w_vq_{p}"][:, :],
                            rhs=ch1[:, cc:cc + GP1, :], start=True, stop=True)
                    nc.scalar.copy(
                        out=qlo[:, :, cc:cc + GP1, :],
                        in_=Y_lo[:, :, 0:NF1].rearrange("r p (g w) -> r p g w", w=128))
                    nc.scalar.copy(
                        out=qh1[:, :, cc:cc + GP1, :],
                        in_=Y_h1[:, :, 0:NF1].rearrange("r p (g w) -> r p g w", w=128))

                # ---- level-1 horizontal diffs -----------------------------
                for q, dl in ((qlo, diffL_lo), (qh1, diffL_h1)):
                    nc.vector.tensor_tensor(
                        out=dl[:, :, :, 1:128], in0=q[:, :, :, 0:127],
                        in1=q[:, :, :, 1:128], op=Alu.subtract)

                # ---- finalize ---------------------------------------------
                lowT = outp.tile([P, G, 2, 256], BF16, tag="lowT")
                highT = outp.tile([P, G, 2, 256], BF16, tag="highT")
                h1x = lv1.tile([P, 2, 2, G, 128], BF16, tag="h1x")
                for p in range(2):
                    nc.vector.scalar_tensor_tensor(
                        out=lowT[:, :, p, 0:256:2], in0=diffL_lo[:, p, :, 0:128],
                        scalar=0.25, in1=qlo[:, p], op0=Alu.mult, op1=Alu.add)
                    nc.vector.scalar_tensor_tensor(
                        out=lowT[:, :, p, 1:256:2], in0=diffL_lo[:, p, :, 1:129],
                        scalar=-0.25, in1=qlo[:, p], op0=Alu.mult, op1=Alu.add)
                    nc.vector.scalar_tensor_tensor(
                        out=h1x[:, p, 0], in0=diffL_h1[:, p, :, 0:128],
                        scalar=0.25, in1=qh1[:, p], op0=Alu.mult, op1=Alu.add)
                    nc.vector.scalar_tensor_tensor(
                        out=h1x[:, p, 1], in0=diffL_h1[:, p, :, 1:129],
                        scalar=-0.25, in1=qh1[:, p], op0=Alu.mult, op1=Alu.add)
                    nc.vector.tensor_tensor(
                        out=highT[:, :, p, 0:256:2], in0=h1x[:, p, 0],
                        in1=h0x[:, p, :, 0:256:2], op=Alu.add)
                    nc.vector.tensor_tensor(
                        out=highT[:, :, p, 1:256:2], in0=h1x[:, p, 1],
                        in1=h0x[:, p, :, 1:256:2], op=Alu.add)

                # ---- store (SWDGE: bf16 -> f32 cast during DMA) -----------
                nc.gpsimd.dma_start(
                    out=low_d[c0:c0 + G].rearrange("c r p w -> r c p w"),
                    in_=lowT)
                nc.gpsimd.dma_start(
                    out=high_d[c0:c0 + G].rearrange("c r p w -> r c p w"),
                    in_=highT)

    nc.compile()
    _NC_CACHE[key] = nc
    return nc


# ----------------------------------------------------------------------------
# host entry points
# ----------------------------------------------------------------------------

_RUNNER = None


def _get_runner():
    """Builds (once) a cached sharded jit executable over the 8 cores."""
    global _RUNNER
    if _RUNNER is not None:
        return _RUNNER

    import jax
    from jax.sharding import Mesh, PartitionSpec, NamedSharding
    from jax.experimental.shard_map import shard_map
    import concourse.mybir as mybir
    from concourse import bass2jax
    from concourse.bass2jax import _bass_exec_p, partition_id_tensor

    bass2jax.install_neuronx_cc_hook()
    nc = build_nc(C_)

    partition_name = nc.partition_id_tensor.name if nc.partition_id_tensor else None
    in_names, out_names, out_avals = [], [], []
    for alloc in nc.m.functions[0].allocations:
        if not isinstance(alloc, mybir.MemoryLocationSet):
            continue
        name = alloc.memorylocations[0].name
        if alloc.kind == "ExternalInput":
            if name != partition_name:
                in_names.append(name)
        elif alloc.kind == "ExternalOutput":
            out_names.append(name)
            out_avals.append(jax.core.ShapedArray(
                tuple(alloc.tensor_shape), mybir.dt.np(alloc.dtype)))
    n_params = len(in_names)
    all_in_names = list(in_names) + list(out_names)
    if partition_name is not None:
        all_in_names.append(partition_name)

    def _body(*args):
        operands = list(args)
        if partition_name is not None:
            operands.append(partition_id_tensor())
        return tuple(_bass_exec_p.bind(
            *operands,
            out_avals=tuple(out_avals),
            in_names=tuple(all_in_names),
            out_names=tuple(out_names),
            lowering_input_output_aliases=(),
            sim_require_finite=True,
            sim_require_nnan=True,
            nc=nc,
        ))

    devices = jax.devices()[:NCORES]
    mesh = Mesh(np.asarray(devices), ("core",))
    n_in = n_params + len(out_names)
    sharded = jax.jit(shard_map(
        _body, mesh=mesh,
        in_specs=(PartitionSpec("core"),) * n_in,
        out_specs=(PartitionSpec("core"),) * len(out_names),
        check_rep=False))

    shard0 = NamedSharding(mesh, PartitionSpec("core"))
    wt = _weights()
    static = {}
    for name in in_names:
        if name == "x":
            continue
        arr = np.concatenate([wt[name]] * NCORES, axis=0)
        static[name] = jax.device_put(arr, shard0)
    for name, aval in zip(out_names, out_avals):
        z = np.zeros((aval.shape[0] * NCORES,) + tuple(aval.shape[1:]),
                     dtype=aval.dtype)
        static[name] = jax.device_put(z, shard0)

    def run(x_global):
        ops = []
        for name in in_names:
            ops.append(x_global if name == "x" else static[name])
        for name in out_names:
            ops.append(static[name])
        outs = sharded(*ops)
        return dict(zip(out_names, outs))

    _RUNNER = (run, shard0)
    return _RUNNER


def _run_device(x, trace=False):
    """x: [8, 64, 256, 256] fp32. Returns (low, high, results_obj)."""
    if trace:
        from concourse import bass_utils
        nc = build_nc(C_)
        wt = _weights()
        in_maps = [
            dict(wt, x=np.ascontiguousarray(x[b]).reshape(C_, 128, 2, W_))
            for b in range(NCORES)
        ]
        res = bass_utils.run_bass_kernel_spmd(
            nc, in_maps, core_ids=list(range(NCORES)), trace=True)
        low = np.stack([
            res.results[b]["low"].reshape(C_, H_, W_) for b in range(NCORES)])
        high = np.stack([
            res.results[b]["high"].reshape(C_, H_, W_) for b in range(NCORES)])
        return low, high, res

    run, _ = _get_runner()
    outs = run(np.ascontiguousarray(x).reshape(B_ * C_, 128, 2, W_))
    low = np.asarray(outs["low"]).reshape(B_, C_, H_, W_)
    high = np.asarray(outs["high"]).reshape(B_, C_, H_, W_)
    return low, high, None


def _fallback(x, level):
    """Numpy port of the reference for unexpected shapes/levels."""
    xl = x.astype(np.float64)
    low = xl
    high = np.zeros_like(xl)
    Bb, Cc, H, W = xl.shape

    def up(a, n_r, n_c):
        Mr = _resize_matrix(a.shape[-2], n_r)
        Mc = _resize_matrix(a.shape[-1], n_c)
        return np.einsum("ij,...jk,lk->...il", Mr, a, Mc)

    for lv in range(level):
        stride = 2 ** lv
        if H // stride < 2 or W // stride < 2:
            break
        x00 = low[..., 0:H - 1:stride, 0:W - 1:stride]
        x01 = low[..., 0:H - 1:stride, 1:W:stride]
        x10 = low[..., 1:H:stride, 0:W - 1:stride]
        x11 = low[..., 1:H:stride, 1:W:stride]
        ll = (x00 + x01 + x10 + x11) * 0.25
        lh = (x00 + x01 - x10 - x11) * 0.25
        hl = (x00 - x01 + x10 - x11) * 0.25
        hh = (x00 - x01 - x10 + x11) * 0.25
        ch = np.abs(lh) + np.abs(hl) + np.abs(hh)
        high = high + up(ch, H, W)
        low = up(ll, H, W)
    if level > 0:
        high = high / level
    return low.astype(np.float32), high.astype(np.float32)


def kernel(x, level):
    x = np.asarray(x, dtype=np.float32)
    level = int(level)
    if level != 2 or x.shape != (B_, C_, H_, W_):
        return _fallback(x, level)
    low, high, _ = _run_device(x)
    return low, high


# revision 10
# speedup vs baseline: 1.4040x; 1.1717x over previous
"""HaarWavelet2D (level=2) Trainium2 kernel, v2.

Contract: kernel(x, level) with x [8, 64, 256, 256] fp32, level=2.
Returns (low_freq, high_freq), each [8, 64, 256, 256] fp32 — matching the
jax reference (2-level Haar decomposition with bilinear resizes).

Sharding: data-parallel over batch — core b processes x[b] (64 channels).

v2 layout (validated in model2.py): rows-in-partitions, row-PARITY tiles
everywhere (row = 2*partition + p).  DRAM tensors are declared
[C, 128, 2, 256] so each G-channel group moves with ONE large DMA
(2 KiB descriptors).  Per group:
  s/d   = column-pair sum/diff, both parities in one TT (f32 in, bf16 out)
  ad    = |d| on the scalar engine; row-shifted (sE, adE) via one
          SBUF->SBUF DMA of an interleaved (s,ad) tile
  t1/m/ch0 = row-pair ops, bf16 TT
  vertical resizes = TensorE matmuls (parity-split banded matrices) in
          Gp=2 chunks, PSUM drained to bf16 SBUF by the scalar engine
  horizontal 255->256 = direct misaligned-bf16 TT diff + W0 mult + add
  level-1 repeats at half size; 128->256 horizontal via diffL tile and
  scalar_tensor_tensor writes straight into the f32 output tiles
Engine assignment (vector/scalar/gpsimd) per op class is parameterized
for rebalancing.
"""

import sys

if "/opt/trn_rl_repo" not in sys.path:
    sys.path.insert(0, "/opt/trn_rl_repo")

import numpy as np
import ml_dtypes

BF = ml_dtypes.bfloat16

B_, C_, H_, W_ = 8, 64, 256, 256
NCORES = 8
G = 4            # channels per group
GP0 = 2          # channels per level-0 matmul chunk
GP1 = 4          # channels per level-1 matmul chunk


# ----------------------------------------------------------------------------
# host-side weight construction
# ----------------------------------------------------------------------------

def _resize_matrix(n, N):
    M = np.zeros((N, n), dtype=np.float64)
    for i in range(N):
        c = (i + 0.5) * n / N - 0.5
        j0 = int(np.floor(c))
        f = c - j0
        M[i, min(max(j0, 0), n - 1)] += 1.0 - f
        M[i, min(max(j0 + 1, 0), n - 1)] += f
    return M


def _build_weights():
    V255 = _resize_matrix(255, 256)
    V128 = _resize_matrix(128, 256)
    Sv1 = np.zeros((255, 256))
    for r in range(255):
        Sv1[r, r] = 1.0
        Sv1[r, r + 1] = 1.0
    Va = 0.25 * (V255 @ Sv1)      # [256, 256]
    Vh = 0.25 * V255              # [256, 255]
    Vq = 0.25 * V128              # [256, 128]
    W0 = np.array([V255[i, i - 1] if i >= 1 else 0.0 for i in range(256)])

    w = {}
    for p in range(2):
        w[f"w_va_{p}e"] = Va[p::2, 0::2].T        # [128,128]
        w[f"w_va_{p}o"] = Va[p::2, 1::2].T        # [128,128]
        w[f"w_vh_{p}e"] = Vh[p::2, 0::2].T        # [128,128]
        w[f"w_vh_{p}o"] = Vh[p::2, 1::2].T        # [127,128]
        w[f"w_vq_{p}"] = Vq[p::2, :].T            # [128,128]
    w["w0t"] = np.tile(W0[None, :], (128, 1))     # [128,256]
    return {k: v.astype(BF) for k, v in w.items()}


_WEIGHTS = None


def _weights():
    global _WEIGHTS
    if _WEIGHTS is None:
        _WEIGHTS = _build_weights()
    return _WEIGHTS


# ----------------------------------------------------------------------------
# bass program
# ----------------------------------------------------------------------------

_NC_CACHE = {}


def build_nc(C=C_):
    key = C
    if key in _NC_CACHE:
        return _NC_CACHE[key]

    import concourse.bass as bass
    import concourse.bacc as bacc
    import concourse.tile as tile
    import concourse.mybir as mybir

    F32 = mybir.dt.float32
    BF16 = mybir.dt.bfloat16
    Alu = mybir.AluOpType
    Act = mybir.ActivationFunctionType
    P = 128

    nc = bacc.Bacc("TRN2", target_bir_lowering=False)
    # [C, 256, 256] viewed as [C, 128, 2, 256]: row = 2r + p
    x_d = nc.dram_tensor("x", [C, P, 2, W_], F32, kind="ExternalInput")
    wt = _weights()
    w_d = {
        name: nc.dram_tensor(name, list(arr.shape), BF16, kind="ExternalInput")
        for name, arr in wt.items()
    }
    low_d = nc.dram_tensor("low", [C, P, 2, W_], F32, kind="ExternalOutput")
    high_d = nc.dram_tensor("high", [C, P, 2, W_], F32, kind="ExternalOutput")

    with tile.TileContext(nc) as tc:
        with (
            tc.tile_pool(name="consts", bufs=1) as consts,
            tc.tile_pool(name="xin", bufs=2) as xin,
            tc.tile_pool(name="front", bufs=2) as front,
            tc.tile_pool(name="qp", bufs=1) as qp,
            tc.tile_pool(name="hor", bufs=2) as hor,
            tc.tile_pool(name="lv1", bufs=2) as lv1,
            tc.tile_pool(name="outp", bufs=2) as outp,
            tc.tile_pool(name="ps0", bufs=1, space="PSUM") as ps0,
        ):
            wtile = {}
            for name, arr in wt.items():
                t = consts.tile(list(arr.shape), BF16, tag=name)
                nc.sync.dma_start(out=t, in_=w_d[name][:, :])
                wtile[name] = t

            # persistent tiles (bufs=1 qp pool -> stable addresses for the
            # one-time edge memsets; values cancel algebraically, the memset
            # only guards against NaN garbage)
            qL = qp.tile([P, 2, G, 256], BF16, tag="qL")
            qh = qp.tile([P, 2, G, 256], BF16, tag="qh")
            diffT_L = qp.tile([P, 2, G, 256], BF16, tag="diffT_L")
            diffT_h = qp.tile([P, 2, G, 256], BF16, tag="diffT_h")
            qlo = qp.tile([P, 2, G, 128], BF16, tag="qlo")
            qh1 = qp.tile([P, 2, G, 128], BF16, tag="qh1")
            diffL_lo = qp.tile([P, 2, G, 129], BF16, tag="diffL_lo")
            diffL_h1 = qp.tile([P, 2, G, 129], BF16, tag="diffL_h1")

            nc.vector.memset(qL[:, :, :, 255:256], 0.0)
            nc.vector.memset(qh[:, :, :, 255:256], 0.0)
            nc.vector.memset(diffT_L[:, :, :, 0:1], 0.0)
            nc.vector.memset(diffT_h[:, :, :, 0:1], 0.0)
            for dl in (diffL_lo, diffL_h1):
                nc.vector.memset(dl[:, :, :, 0:1], 0.0)
                nc.vector.memset(dl[:, :, :, 128:129], 0.0)

            w0b = wtile["w0t"][:, :]
            w0_ap = bass.AP(tensor=w0b.tensor, offset=w0b.offset,
                            ap=[w0b.ap[0], [0, 2], [0, G], w0b.ap[1]])

            n_iter = C // G
            for it in range(n_iter):
                c0 = it * G

                # ---- load -------------------------------------------------
                X = xin.tile([P, G, 2, 256], BF16, tag="X")
                nc.gpsimd.dma_start(
                    out=X, in_=x_d[c0:c0 + G].rearrange("c r p w -> r c p w"))

                # ---- level-0 column pairs (parity-first outputs) ----------
                SDA = front.tile([P, 4, G, 255], BF16, tag="SDA")
                d0 = front.tile([P, 2, G, 255], BF16, tag="d0")
                Xp = X[:, :, :, 0:255].rearrange("r g p w -> r p g w")
                Xp1 = X[:, :, :, 1:256].rearrange("r g p w -> r p g w")
                nc.vector.tensor_tensor(
                    out=SDA[:, 0:2], in0=Xp, in1=Xp1, op=Alu.add)
                nc.vector.tensor_tensor(
                    out=d0, in0=Xp, in1=Xp1, op=Alu.subtract)
                nc.scalar.activation(out=SDA[:, 2:4], in_=d0, func=Act.Abs)

                SDA2 = front.tile([127, 2, G, 255], BF16, tag="SDA2")
                nc.sync.dma_start(out=SDA2, in_=SDA[1:128, 0:4:2])

                # ---- level-0 row pairs ------------------------------------
                t1 = front.tile([P, 2, G, 255], BF16, tag="t1")
                m0 = front.tile([P, 2, G, 255], BF16, tag="m0")
                a1 = front.tile([P, 2, G, 255], BF16, tag="a1")
                ch0 = front.tile([P, 2, G, 255], BF16, tag="ch0")
                nc.vector.tensor_tensor(
                    out=t1[:, 0], in0=SDA[:, 0], in1=SDA[:, 1], op=Alu.subtract)
                nc.vector.tensor_tensor(
                    out=t1[0:127, 1], in0=SDA[0:127, 1], in1=SDA2[:, 0],
                    op=Alu.subtract)
                nc.vector.tensor_tensor(
                    out=m0[:, 0], in0=SDA[:, 2], in1=SDA[:, 3], op=Alu.max)
                nc.vector.tensor_tensor(
                    out=m0[0:127, 1], in0=SDA[0:127, 3], in1=SDA2[:, 1],
                    op=Alu.max)
                nc.scalar.activation(
                    out=a1[:, 0], in_=t1[:, 0], func=Act.Abs, scale=0.5)
                nc.scalar.activation(
                    out=a1[0:127, 1], in_=t1[0:127, 1], func=Act.Abs, scale=0.5)
                nc.vector.tensor_tensor(
                    out=ch0[:, 0], in0=a1[:, 0], in1=m0[:, 0], op=Alu.add)
                nc.vector.tensor_tensor(
                    out=ch0[0:127, 1], in0=a1[0:127, 1], in1=m0[0:127, 1],
                    op=Alu.add)

                # ---- level-0 vertical matmuls + drains --------------------
                # one PSUM round per group; matmul free dim capped at 510
                NF0 = GP0 * 255
                NCH = G // GP0
                Y_L = ps0.tile([P, 2, NCH, 512], F32, tag="Y_A")
                Y_h = ps0.tile([P, 2, NCH, 512], F32, tag="Y_B")
                for p in range(2):
                    for ci in range(NCH):
                        cc = ci * GP0
                        nc.tensor.matmul(
                            out=Y_L[:, p, ci, 0:NF0], lhsT=wtile[f"w_va_{p}e"][:, :],
                            rhs=SDA[:, 0, cc:cc + GP0, :], start=True, stop=False)
                        nc.tensor.matmul(
                            out=Y_L[:, p, ci, 0:NF0], lhsT=wtile[f"w_va_{p}o"][:, :],
                            rhs=SDA[:, 1, cc:cc + GP0, :], start=False, stop=True)
                        nc.tensor.matmul(
                            out=Y_h[:, p, ci, 0:NF0], lhsT=wtile[f"w_vh_{p}e"][:, :],
                            rhs=ch0[:, 0, cc:cc + GP0, :], start=True, stop=False)
                        nc.tensor.matmul(
                            out=Y_h[:, p, ci, 0:NF0], lhsT=wtile[f"w_vh_{p}o"][:, :],
                            rhs=ch0[0:127, 1, cc:cc + GP0, :], start=False,
                            stop=True)
                for ci in range(NCH):
                    cc = ci * GP0
                    nc.scalar.copy(
                        out=qL[:, :, cc:cc + GP0, 0:255],
                        in_=Y_L[:, :, ci, 0:NF0].rearrange(
                            "r p (g w) -> r p g w", w=255))
                    nc.scalar.copy(
                        out=qh[:, :, cc:cc + GP0, 0:255],
                        in_=Y_h[:, :, ci, 0:NF0].rearrange(
                            "r p (g w) -> r p g w", w=255))

                # ---- level-0 horizontal resize (255->256) -----------------
                L0x = hor.tile([P, 2, G, 256], BF16, tag="L0x")
                h0x = hor.tile([P, 2, G, 256], BF16, tag="h0x")
                tmpT = hor.tile([P, 2, G, 256], BF16, tag="tmpT")
                for q, diffT, out in (
                    (qL, diffT_L, L0x),
                    (qh, diffT_h, h0x),
                ):
                    nc.vector.tensor_tensor(
                        out=diffT[:, :, :, 1:256], in0=q[:, :, :, 0:255],
                        in1=q[:, :, :, 1:256], op=Alu.subtract)
                    nc.vector.tensor_tensor(
                        out=tmpT, in0=diffT, in1=w0_ap, op=Alu.mult)
                    nc.vector.tensor_tensor(out=out, in0=q, in1=tmpT, op=Alu.add)

                # ---- level-1 ----------------------------------------------
                s2 = lv1.tile([P, 2, G, 128], BF16, tag="s2")
                d2 = lv1.tile([P, 2, G, 128], BF16, tag="d2")
                ad2 = lv1.tile([P, 2, G, 128], BF16, tag="ad2")
                nc.gpsimd.tensor_tensor(
                    out=s2, in0=L0x[:, :, :, 0:256:2], in1=L0x[:, :, :, 1:256:2],
                    op=Alu.add)
                nc.gpsimd.tensor_tensor(
                    out=d2, in0=L0x[:, :, :, 0:256:2], in1=L0x[:, :, :, 1:256:2],
                    op=Alu.subtract)
                nc.scalar.activation(out=ad2, in_=d2, func=Act.Abs)
                lsum1 = lv1.tile([P, G, 128], BF16, tag="lsum1")
                t1b = lv1.tile([P, G, 128], BF16, tag="t1b")
                m1 = lv1.tile([P, G, 128], BF16, tag="m1")
                a1b = lv1.tile([P, G, 128], BF16, tag="a1b")
                ch1 = lv1.tile([P, G, 128], BF16, tag="ch1")
                nc.gpsimd.tensor_tensor(
                    out=lsum1, in0=s2[:, 0], in1=s2[:, 1], op=Alu.add)
                nc.gpsimd.tensor_tensor(
                    out=t1b, in0=s2[:, 0], in1=s2[:, 1], op=Alu.subtract)
                nc.vector.tensor_tensor(
                    out=m1, in0=ad2[:, 0], in1=ad2[:, 1], op=Alu.max)
                nc.scalar.activation(out=a1b, in_=t1b, func=Act.Abs, scale=0.5)
                nc.vector.tensor_tensor(
                    out=ch1, in0=a1b, in1=m1, op=Alu.add)

                # ---- level-1 vertical matmuls + drains --------------------
                NF1 = G * 128
                Y_lo = ps0.tile([P, 2, NCH, 512], F32, tag="Y_A")
                Y_h1 = ps0.tile([P, 2, NCH, 512], F32, tag="Y_B")
                for p in range(2):
                    nc.tensor.matmul(
                        out=Y_lo[:, p, 0, 0:NF1], lhsT=wtile[f"w_vq_{p}"][:, :],
                        rhs=lsum1, start=True, stop=True)
                    nc.tensor.matmul(
                        out=Y_h1[:, p, 0, 0:NF1], lhsT=wtile[f"w_vq_{p}"][:, :],
                        rhs=ch1, start=True, stop=True)
                nc.scalar.copy(
                    out=qlo,
                    in_=Y_lo[:, :, 0, 0:NF1].rearrange("r p (g w) -> r p g w", w=128))
                nc.scalar.copy(
                    out=qh1,
                    in_=Y_h1[:, :, 0, 0:NF1].rearrange("r p (g w) -> r p g w", w=128))

                # ---- level-1 horizontal diffs -----------------------------
                for q, dl in ((qlo, diffL_lo), (qh1, diffL_h1)):
                    nc.vector.tensor_tensor(
                        out=dl[:, :, :, 1:128], in0=q[:, :, :, 0:127],
                        in1=q[:, :, :, 1:128], op=Alu.subtract)

                # ---- finalize ---------------------------------------------
                lowT = outp.tile([P, G, 2, 256], BF16, tag="lowT")
                highT = outp.tile([P, G, 2, 256], BF16, tag="highT")
                hh = lv1.tile([P, 2, G, 256], BF16, tag="hh")
                for p in range(2):
                    nc.vector.scalar_tensor_tensor(
                        out=lowT[:, :, p, 0:256:2], in0=diffL_lo[:, p, :, 0:128],
                        scalar=0.25, in1=qlo[:, p], op0=Alu.mult, op1=Alu.add)
                    nc.vector.scalar_tensor_tensor(
                        out=lowT[:, :, p, 1:256:2], in0=diffL_lo[:, p, :, 1:129],
                        scalar=-0.25, in1=qlo[:, p], op0=Alu.mult, op1=Alu.add)
                    nc.vector.scalar_tensor_tensor(
                        out=hh[:, p, :, 0:256:2], in0=diffL_h1[:, p, :, 0:128],
                        scalar=0.25, in1=qh1[:, p], op0=Alu.mult, op1=Alu.add)
                    nc.vector.scalar_tensor_tensor(
                        out=hh[:, p, :, 1:256:2], in0=diffL_h1[:, p, :, 1:129],
                        scalar=-0.25, in1=qh1[:, p], op0=Alu.mult, op1=Alu.add)
                nc.vector.tensor_tensor(
                    out=highT.rearrange("r c p w -> r p c w"), in0=hh, in1=h0x,
                    op=Alu.add)

                # ---- store (SWDGE: bf16 -> f32 cast during DMA) -----------
                nc.gpsimd.dma_start(
                    out=low_d[c0:c0 + G].rearrange("c r p w -> r c p w"),
                    in_=lowT)
                nc.gpsimd.dma_start(
                    out=high_d[c0:c0 + G].rearrange("c r p w -> r c p w"),
                    in_=highT)

    nc.compile()
    _NC_CACHE[key] = nc
    return nc


# ----------------------------------------------------------------------------
# host entry points
# ----------------------------------------------------------------------------

_RUNNER = None


def _get_runner():
    """Builds (once) a cached sharded jit executable over the 8 cores."""
    global _RUNNER
    if _RUNNER is not None:
        return _RUNNER

    import jax
    from jax.sharding import Mesh, PartitionSpec, NamedSharding
    from jax.experimental.shard_map import shard_map
    import concourse.mybir as mybir
    from concourse import bass2jax
    from concourse.bass2jax import _bass_exec_p, partition_id_tensor

    bass2jax.install_neuronx_cc_hook()
    nc = build_nc(C_)

    partition_name = nc.partition_id_tensor.name if nc.partition_id_tensor else None
    in_names, out_names, out_avals = [], [], []
    for alloc in nc.m.functions[0].allocations:
        if not isinstance(alloc, mybir.MemoryLocationSet):
            continue
        name = alloc.memorylocations[0].name
        if alloc.kind == "ExternalInput":
            if name != partition_name:
                in_names.append(name)
        elif alloc.kind == "ExternalOutput":
            out_names.append(name)
            out_avals.append(jax.core.ShapedArray(
                tuple(alloc.tensor_shape), mybir.dt.np(alloc.dtype)))
    n_params = len(in_names)
    all_in_names = list(in_names) + list(out_names)
    if partition_name is not None:
        all_in_names.append(partition_name)

    def _body(*args):
        operands = list(args)
        if partition_name is not None:
            operands.append(partition_id_tensor())
        return tuple(_bass_exec_p.bind(
            *operands,
            out_avals=tuple(out_avals),
            in_names=tuple(all_in_names),
            out_names=tuple(out_names),
            lowering_input_output_aliases=(),
            sim_require_finite=True,
            sim_require_nnan=True,
            nc=nc,
        ))

    devices = jax.devices()[:NCORES]
    mesh = Mesh(np.asarray(devices), ("core",))
    n_in = n_params + len(out_names)
    sharded = jax.jit(shard_map(
        _body, mesh=mesh,
        in_specs=(PartitionSpec("core"),) * n_in,
        out_specs=(PartitionSpec("core"),) * len(out_names),
        check_rep=False))

    shard0 = NamedSharding(mesh, PartitionSpec("core"))
    wt = _weights()
    static = {}
    for name in in_names:
        if name == "x":
            continue
        arr = np.concatenate([wt[name]] * NCORES, axis=0)
        static[name] = jax.device_put(arr, shard0)
    for name, aval in zip(out_names, out_avals):
        z = np.zeros((aval.shape[0] * NCORES,) + tuple(aval.shape[1:]),
                     dtype=aval.dtype)
        static[name] = jax.device_put(z, shard0)

    def run(x_global):
        ops = []
        for name in in_names:
            ops.append(x_global if name == "x" else static[name])
        for name in out_names:
            ops.append(static[name])
        outs = sharded(*ops)
        return dict(zip(out_names, outs))

    _RUNNER = (run, shard0)
    return _RUNNER


def _run_device(x, trace=False):
    """x: [8, 64, 256, 256] fp32. Returns (low, high, results_obj)."""
    if trace:
        from concourse import bass_utils
        nc = build_nc(C_)
        wt = _weights()
        in_maps = [
            dict(wt, x=np.ascontiguousarray(x[b]).reshape(C_, 128, 2, W_))
            for b in range(NCORES)
        ]
        res = bass_utils.run_bass_kernel_spmd(
            nc, in_maps, core_ids=list(range(NCORES)), trace=True)
        low = np.stack([
            res.results[b]["low"].reshape(C_, H_, W_) for b in range(NCORES)])
        high = np.stack([
            res.results[b]["high"].reshape(C_, H_, W_) for b in range(NCORES)])
        return low, high, res

    run, _ = _get_runner()
    outs = run(np.ascontiguousarray(x).reshape(B_ * C_, 128, 2, W_))
    low = np.asarray(outs["low"]).reshape(B_, C_, H_, W_)
    high = np.asarray(outs["high"]).reshape(B_, C_, H_, W_)
    return low, high, None


def _fallback(x, level):
    """Numpy port of the reference for unexpected shapes/levels."""
    xl = x.astype(np.float64)
    low = xl
    high = np.zeros_like(xl)
    Bb, Cc, H, W = xl.shape

    def up(a, n_r, n_c):
        Mr = _resize_matrix(a.shape[-2], n_r)
        Mc = _resize_matrix(a.shape[-1], n_c)
        return np.einsum("ij,...jk,lk->...il", Mr, a, Mc)

    for lv in range(level):
        stride = 2 ** lv
        if H // stride < 2 or W // stride < 2:
            break
        x00 = low[..., 0:H - 1:stride, 0:W - 1:stride]
        x01 = low[..., 0:H - 1:stride, 1:W:stride]
        x10 = low[..., 1:H:stride, 0:W - 1:stride]
        x11 = low[..., 1:H:stride, 1:W:stride]
        ll = (x00 + x01 + x10 + x11) * 0.25
        lh = (x00 + x01 - x10 - x11) * 0.25
        hl = (x00 - x01 + x10 - x11) * 0.25
        hh = (x00 - x01 - x10 + x11) * 0.25
        ch = np.abs(lh) + np.abs(hl) + np.abs(hh)
        high = high + up(ch, H, W)
        low = up(ll, H, W)
    if level > 0:
        high = high / level
    return low.astype(np.float32), high.astype(np.float32)


def kernel(x, level):
    x = np.asarray(x, dtype=np.float32)
    level = int(level)
    if level != 2 or x.shape != (B_, C_, H_, W_):
        return _fallback(x, level)
    low, high, _ = _run_device(x)
    return low, high


# revision 11
# speedup vs baseline: 1.5088x; 1.0746x over previous
"""HaarWavelet2D (level=2) Trainium2 kernel, v2.

Contract: kernel(x, level) with x [8, 64, 256, 256] fp32, level=2.
Returns (low_freq, high_freq), each [8, 64, 256, 256] fp32 — matching the
jax reference (2-level Haar decomposition with bilinear resizes).

Sharding: data-parallel over batch — core b processes x[b] (64 channels).

v2 layout (validated in model2.py): rows-in-partitions, row-PARITY tiles
everywhere (row = 2*partition + p).  DRAM tensors are declared
[C, 128, 2, 256] so each G-channel group moves with ONE large DMA
(2 KiB descriptors).  Per group:
  s/d   = column-pair sum/diff, both parities in one TT (f32 in, bf16 out)
  ad    = |d| on the scalar engine; row-shifted (sE, adE) via one
          SBUF->SBUF DMA of an interleaved (s,ad) tile
  t1/m/ch0 = row-pair ops, bf16 TT
  vertical resizes = TensorE matmuls (parity-split banded matrices) in
          Gp=2 chunks, PSUM drained to bf16 SBUF by the scalar engine
  horizontal 255->256 = direct misaligned-bf16 TT diff + W0 mult + add
  level-1 repeats at half size; 128->256 horizontal via diffL tile and
  scalar_tensor_tensor writes straight into the f32 output tiles
Engine assignment (vector/scalar/gpsimd) per op class is parameterized
for rebalancing.
"""

import sys

if "/opt/trn_rl_repo" not in sys.path:
    sys.path.insert(0, "/opt/trn_rl_repo")

import numpy as np
import ml_dtypes

BF = ml_dtypes.bfloat16

B_, C_, H_, W_ = 8, 64, 256, 256
NCORES = 8
G = 4            # channels per group
GP0 = 2          # channels per level-0 matmul chunk
GP1 = 4          # channels per level-1 matmul chunk


# ----------------------------------------------------------------------------
# host-side weight construction
# ----------------------------------------------------------------------------

def _resize_matrix(n, N):
    M = np.zeros((N, n), dtype=np.float64)
    for i in range(N):
        c = (i + 0.5) * n / N - 0.5
        j0 = int(np.floor(c))
        f = c - j0
        M[i, min(max(j0, 0), n - 1)] += 1.0 - f
        M[i, min(max(j0 + 1, 0), n - 1)] += f
    return M


def _build_weights():
    V255 = _resize_matrix(255, 256)
    V128 = _resize_matrix(128, 256)
    Sv1 = np.zeros((255, 256))
    for r in range(255):
        Sv1[r, r] = 1.0
        Sv1[r, r + 1] = 1.0
    Va = 0.25 * (V255 @ Sv1)      # [256, 256]
    Vh = 0.25 * V255              # [256, 255]
    Vq = 0.25 * V128              # [256, 128]
    W0 = np.array([V255[i, i - 1] if i >= 1 else 0.0 for i in range(256)])

    w = {}
    for p in range(2):
        w[f"w_va_{p}e"] = Va[p::2, 0::2].T        # [128,128]
        w[f"w_va_{p}o"] = Va[p::2, 1::2].T        # [128,128]
        w[f"w_vh_{p}e"] = Vh[p::2, 0::2].T        # [128,128]
        w[f"w_vh_{p}o"] = Vh[p::2, 1::2].T        # [127,128]
        w[f"w_vq_{p}"] = Vq[p::2, :].T            # [128,128]
    w["w0t"] = np.tile(W0[None, :], (128, 1))     # [128,256]
    return {k: v.astype(BF) for k, v in w.items()}


_WEIGHTS = None


def _weights():
    global _WEIGHTS
    if _WEIGHTS is None:
        _WEIGHTS = _build_weights()
    return _WEIGHTS


# ----------------------------------------------------------------------------
# bass program
# ----------------------------------------------------------------------------

_NC_CACHE = {}


def build_nc(C=C_):
    key = C
    if key in _NC_CACHE:
        return _NC_CACHE[key]

    import concourse.bass as bass
    import concourse.bacc as bacc
    import concourse.tile as tile
    import concourse.mybir as mybir

    F32 = mybir.dt.float32
    BF16 = mybir.dt.bfloat16
    Alu = mybir.AluOpType
    Act = mybir.ActivationFunctionType
    P = 128

    nc = bacc.Bacc("TRN2", target_bir_lowering=False)
    # [C, 256, 256] viewed as [C, 128, 2, 256]: row = 2r + p
    x_d = nc.dram_tensor("x", [C, P, 2, W_], F32, kind="ExternalInput")
    wt = _weights()
    w_d = {
        name: nc.dram_tensor(name, list(arr.shape), BF16, kind="ExternalInput")
        for name, arr in wt.items()
    }
    low_d = nc.dram_tensor("low", [C, P, 2, W_], F32, kind="ExternalOutput")
    high_d = nc.dram_tensor("high", [C, P, 2, W_], F32, kind="ExternalOutput")

    with tile.TileContext(nc) as tc:
        with (
            tc.tile_pool(name="consts", bufs=1) as consts,
            tc.tile_pool(name="xin", bufs=2) as xin,
            tc.tile_pool(name="front", bufs=2) as front,
            tc.tile_pool(name="qp", bufs=1) as qp,
            tc.tile_pool(name="hor", bufs=2) as hor,
            tc.tile_pool(name="lv1", bufs=2) as lv1,
            tc.tile_pool(name="outp", bufs=2) as outp,
            tc.tile_pool(name="ps0", bufs=1, space="PSUM") as ps0,
        ):
            wtile = {}
            for name, arr in wt.items():
                t = consts.tile(list(arr.shape), BF16, tag=name)
                nc.sync.dma_start(out=t, in_=w_d[name][:, :])
                wtile[name] = t

            # persistent tiles (bufs=1 qp pool -> stable addresses for the
            # one-time edge memsets; values cancel algebraically, the memset
            # only guards against NaN garbage)
            qL2 = qp.tile([P, 2, 2, G, 256], BF16, tag="qL")
            qh2 = qp.tile([P, 2, 2, G, 256], BF16, tag="qh")
            diffT_L2 = qp.tile([P, 2, 2, G, 256], BF16, tag="diffT_L")
            diffT_h2 = qp.tile([P, 2, 2, G, 256], BF16, tag="diffT_h")
            qlo2 = qp.tile([P, 2, 2, G, 128], BF16, tag="qlo")
            qh12 = qp.tile([P, 2, 2, G, 128], BF16, tag="qh1")
            diffL_lo2 = qp.tile([P, 2, 2, G, 129], BF16, tag="diffL_lo")
            diffL_h12 = qp.tile([P, 2, 2, G, 129], BF16, tag="diffL_h1")

            nc.vector.memset(qL2[:, :, :, :, 255:256], 0.0)
            nc.vector.memset(qh2[:, :, :, :, 255:256], 0.0)
            nc.vector.memset(diffT_L2[:, :, :, :, 0:1], 0.0)
            nc.vector.memset(diffT_h2[:, :, :, :, 0:1], 0.0)
            for dl in (diffL_lo2, diffL_h12):
                nc.vector.memset(dl[:, :, :, :, 0:1], 0.0)
                nc.vector.memset(dl[:, :, :, :, 128:129], 0.0)

            w0b = wtile["w0t"][:, :]
            w0_ap = bass.AP(tensor=w0b.tensor, offset=w0b.offset,
                            ap=[w0b.ap[0], [0, 2], [0, G], w0b.ap[1]])

            n_iter = C // G
            for it in range(n_iter):
                c0 = it * G
                sl = it % 2
                qL, qh = qL2[:, sl], qh2[:, sl]
                diffT_L, diffT_h = diffT_L2[:, sl], diffT_h2[:, sl]
                qlo, qh1 = qlo2[:, sl], qh12[:, sl]
                diffL_lo, diffL_h1 = diffL_lo2[:, sl], diffL_h12[:, sl]

                # ---- load -------------------------------------------------
                X = xin.tile([P, G, 2, 256], BF16, tag="X")
                nc.gpsimd.dma_start(
                    out=X, in_=x_d[c0:c0 + G].rearrange("c r p w -> r c p w"))

                # ---- level-0 column pairs (parity-first outputs) ----------
                SDA = front.tile([P, 4, G, 255], BF16, tag="SDA")
                d0 = front.tile([P, 2, G, 255], BF16, tag="d0")
                Xp = X[:, :, :, 0:255].rearrange("r g p w -> r p g w")
                Xp1 = X[:, :, :, 1:256].rearrange("r g p w -> r p g w")
                nc.vector.tensor_tensor(
                    out=SDA[:, 0:2], in0=Xp, in1=Xp1, op=Alu.add)
                nc.vector.tensor_tensor(
                    out=d0, in0=Xp, in1=Xp1, op=Alu.subtract)
                nc.scalar.activation(out=SDA[:, 2:4], in_=d0, func=Act.Abs)

                SDA2 = front.tile([127, 2, G, 255], BF16, tag="SDA2")
                nc.sync.dma_start(out=SDA2, in_=SDA[1:128, 0:4:2])

                # ---- level-0 row pairs ------------------------------------
                t1 = front.tile([P, 2, G, 255], BF16, tag="t1")
                m0 = front.tile([P, 2, G, 255], BF16, tag="m0")
                a1 = front.tile([P, 2, G, 255], BF16, tag="a1")
                ch0 = front.tile([P, 2, G, 255], BF16, tag="ch0")
                nc.vector.tensor_tensor(
                    out=t1[:, 0], in0=SDA[:, 0], in1=SDA[:, 1], op=Alu.subtract)
                nc.vector.tensor_tensor(
                    out=t1[0:127, 1], in0=SDA[0:127, 1], in1=SDA2[:, 0],
                    op=Alu.subtract)
                nc.vector.tensor_tensor(
                    out=m0[:, 0], in0=SDA[:, 2], in1=SDA[:, 3], op=Alu.max)
                nc.vector.tensor_tensor(
                    out=m0[0:127, 1], in0=SDA[0:127, 3], in1=SDA2[:, 1],
                    op=Alu.max)
                nc.scalar.activation(
                    out=a1[:, 0], in_=t1[:, 0], func=Act.Abs, scale=0.5)
                nc.scalar.activation(
                    out=a1[0:127, 1], in_=t1[0:127, 1], func=Act.Abs, scale=0.5)
                nc.vector.tensor_tensor(
                    out=ch0[:, 0], in0=a1[:, 0], in1=m0[:, 0], op=Alu.add)
                nc.vector.tensor_tensor(
                    out=ch0[0:127, 1], in0=a1[0:127, 1], in1=m0[0:127, 1],
                    op=Alu.add)

                # ---- level-0 vertical matmuls + drains --------------------
                # one PSUM round per group; matmul free dim capped at 510
                NF0 = GP0 * 255
                NCH = G // GP0
                for ci in range(NCH):
                    cc = ci * GP0
                    Y_L = ps0.tile([P, 2, 512], F32, tag=f"Y_A{ci}")
                    Y_h = ps0.tile([P, 2, 512], F32, tag=f"Y_B{ci}")
                    for p in range(2):
                        nc.tensor.matmul(
                            out=Y_L[:, p, 0:NF0], lhsT=wtile[f"w_va_{p}e"][:, :],
                            rhs=SDA[:, 0, cc:cc + GP0, :], start=True, stop=False)
                        nc.tensor.matmul(
                            out=Y_L[:, p, 0:NF0], lhsT=wtile[f"w_va_{p}o"][:, :],
                            rhs=SDA[:, 1, cc:cc + GP0, :], start=False, stop=True)
                        nc.tensor.matmul(
                            out=Y_h[:, p, 0:NF0], lhsT=wtile[f"w_vh_{p}e"][:, :],
                            rhs=ch0[:, 0, cc:cc + GP0, :], start=True, stop=False)
                        nc.tensor.matmul(
                            out=Y_h[:, p, 0:NF0], lhsT=wtile[f"w_vh_{p}o"][:, :],
                            rhs=ch0[0:127, 1, cc:cc + GP0, :], start=False,
                            stop=True)
                    nc.scalar.copy(
                        out=qL[:, :, cc:cc + GP0, 0:255],
                        in_=Y_L[:, :, 0:NF0].rearrange(
                            "r p (g w) -> r p g w", w=255))
                    nc.scalar.copy(
                        out=qh[:, :, cc:cc + GP0, 0:255],
                        in_=Y_h[:, :, 0:NF0].rearrange(
                            "r p (g w) -> r p g w", w=255))

                # ---- level-0 horizontal resize (255->256) -----------------
                L0x = hor.tile([P, 2, G, 256], BF16, tag="L0x")
                h0x = hor.tile([P, 2, G, 256], BF16, tag="h0x")
                tmpT = hor.tile([P, 2, G, 256], BF16, tag="tmpT")
                for q, diffT, out in (
                    (qL, diffT_L, L0x),
                    (qh, diffT_h, h0x),
                ):
                    nc.vector.tensor_tensor(
                        out=diffT[:, :, :, 1:256], in0=q[:, :, :, 0:255],
                        in1=q[:, :, :, 1:256], op=Alu.subtract)
                    nc.vector.tensor_tensor(
                        out=tmpT, in0=diffT, in1=w0_ap, op=Alu.mult)
                    nc.vector.tensor_tensor(out=out, in0=q, in1=tmpT, op=Alu.add)

                # ---- level-1 ----------------------------------------------
                s2 = lv1.tile([P, 2, G, 128], BF16, tag="s2")
                d2 = lv1.tile([P, 2, G, 128], BF16, tag="d2")
                ad2 = lv1.tile([P, 2, G, 128], BF16, tag="ad2")
                nc.gpsimd.tensor_tensor(
                    out=s2, in0=L0x[:, :, :, 0:256:2], in1=L0x[:, :, :, 1:256:2],
                    op=Alu.add)
                nc.gpsimd.tensor_tensor(
                    out=d2, in0=L0x[:, :, :, 0:256:2], in1=L0x[:, :, :, 1:256:2],
                    op=Alu.subtract)
                nc.scalar.activation(out=ad2, in_=d2, func=Act.Abs)
                lsum1 = lv1.tile([P, G, 128], BF16, tag="lsum1")
                t1b = lv1.tile([P, G, 128], BF16, tag="t1b")
                m1 = lv1.tile([P, G, 128], BF16, tag="m1")
                a1b = lv1.tile([P, G, 128], BF16, tag="a1b")
                ch1 = lv1.tile([P, G, 128], BF16, tag="ch1")
                nc.gpsimd.tensor_tensor(
                    out=lsum1, in0=s2[:, 0], in1=s2[:, 1], op=Alu.add)
                nc.gpsimd.tensor_tensor(
                    out=t1b, in0=s2[:, 0], in1=s2[:, 1], op=Alu.subtract)
                nc.vector.tensor_tensor(
                    out=m1, in0=ad2[:, 0], in1=ad2[:, 1], op=Alu.max)
                nc.scalar.activation(out=a1b, in_=t1b, func=Act.Abs, scale=0.5)
                nc.vector.tensor_tensor(
                    out=ch1, in0=a1b, in1=m1, op=Alu.add)

                # ---- level-1 vertical matmuls + drains --------------------
                NF1 = G * 128
                Y_lo = ps0.tile([P, 2, 512], F32, tag="Y_A0")
                Y_h1 = ps0.tile([P, 2, 512], F32, tag="Y_B0")
                for p in range(2):
                    nc.tensor.matmul(
                        out=Y_lo[:, p, 0:NF1], lhsT=wtile[f"w_vq_{p}"][:, :],
                        rhs=lsum1, start=True, stop=True)
                    nc.tensor.matmul(
                        out=Y_h1[:, p, 0:NF1], lhsT=wtile[f"w_vq_{p}"][:, :],
                        rhs=ch1, start=True, stop=True)
                nc.scalar.copy(
                    out=qlo,
                    in_=Y_lo[:, :, 0:NF1].rearrange("r p (g w) -> r p g w", w=128))
                nc.scalar.copy(
                    out=qh1,
                    in_=Y_h1[:, :, 0:NF1].rearrange("r p (g w) -> r p g w", w=128))

                # ---- level-1 horizontal diffs -----------------------------
                for q, dl in ((qlo, diffL_lo), (qh1, diffL_h1)):
                    nc.vector.tensor_tensor(
                        out=dl[:, :, :, 1:128], in0=q[:, :, :, 0:127],
                        in1=q[:, :, :, 1:128], op=Alu.subtract)

                # ---- finalize ---------------------------------------------
                lowT = outp.tile([P, G, 2, 256], BF16, tag="lowT")
                highT = outp.tile([P, G, 2, 256], BF16, tag="highT")
                hh = lv1.tile([P, 2, G, 256], BF16, tag="hh")
                for p in range(2):
                    nc.vector.scalar_tensor_tensor(
                        out=lowT[:, :, p, 0:256:2], in0=diffL_lo[:, p, :, 0:128],
                        scalar=0.25, in1=qlo[:, p], op0=Alu.mult, op1=Alu.add)
                    nc.vector.scalar_tensor_tensor(
                        out=lowT[:, :, p, 1:256:2], in0=diffL_lo[:, p, :, 1:129],
                        scalar=-0.25, in1=qlo[:, p], op0=Alu.mult, op1=Alu.add)
                    nc.vector.scalar_tensor_tensor(
                        out=hh[:, p, :, 0:256:2], in0=diffL_h1[:, p, :, 0:128],
                        scalar=0.25, in1=qh1[:, p], op0=Alu.mult, op1=Alu.add)
                    nc.vector.scalar_tensor_tensor(
                        out=hh[:, p, :, 1:256:2], in0=diffL_h1[:, p, :, 1:129],
                        scalar=-0.25, in1=qh1[:, p], op0=Alu.mult, op1=Alu.add)
                nc.vector.tensor_tensor(
                    out=highT.rearrange("r c p w -> r p c w"), in0=hh, in1=h0x,
                    op=Alu.add)

                # ---- store (SWDGE: bf16 -> f32 cast during DMA) -----------
                nc.gpsimd.dma_start(
                    out=low_d[c0:c0 + G].rearrange("c r p w -> r c p w"),
                    in_=lowT)
                nc.gpsimd.dma_start(
                    out=high_d[c0:c0 + G].rearrange("c r p w -> r c p w"),
                    in_=highT)

    nc.compile()
    _NC_CACHE[key] = nc
    return nc


# ----------------------------------------------------------------------------
# host entry points
# ----------------------------------------------------------------------------

_RUNNER = None


def _get_runner():
    """Builds (once) a cached sharded jit executable over the 8 cores."""
    global _RUNNER
    if _RUNNER is not None:
        return _RUNNER

    import jax
    from jax.sharding import Mesh, PartitionSpec, NamedSharding
    from jax.experimental.shard_map import shard_map
    import concourse.mybir as mybir
    from concourse import bass2jax
    from concourse.bass2jax import _bass_exec_p, partition_id_tensor

    bass2jax.install_neuronx_cc_hook()
    nc = build_nc(C_)

    partition_name = nc.partition_id_tensor.name if nc.partition_id_tensor else None
    in_names, out_names, out_avals = [], [], []
    for alloc in nc.m.functions[0].allocations:
        if not isinstance(alloc, mybir.MemoryLocationSet):
            continue
        name = alloc.memorylocations[0].name
        if alloc.kind == "ExternalInput":
            if name != partition_name:
                in_names.append(name)
        elif alloc.kind == "ExternalOutput":
            out_names.append(name)
            out_avals.append(jax.core.ShapedArray(
                tuple(alloc.tensor_shape), mybir.dt.np(alloc.dtype)))
    n_params = len(in_names)
    all_in_names = list(in_names) + list(out_names)
    if partition_name is not None:
        all_in_names.append(partition_name)

    def _body(*args):
        operands = list(args)
        if partition_name is not None:
            operands.append(partition_id_tensor())
        return tuple(_bass_exec_p.bind(
            *operands,
            out_avals=tuple(out_avals),
            in_names=tuple(all_in_names),
            out_names=tuple(out_names),
            lowering_input_output_aliases=(),
            sim_require_finite=True,
            sim_require_nnan=True,
            nc=nc,
        ))

    devices = jax.devices()[:NCORES]
    mesh = Mesh(np.asarray(devices), ("core",))
    n_in = n_params + len(out_names)
    sharded = jax.jit(shard_map(
        _body, mesh=mesh,
        in_specs=(PartitionSpec("core"),) * n_in,
        out_specs=(PartitionSpec("core"),) * len(out_names),
        check_rep=False))

    shard0 = NamedSharding(mesh, PartitionSpec("core"))
    wt = _weights()
    static = {}
    for name in in_names:
        if name == "x":
            continue
        arr = np.concatenate([wt[name]] * NCORES, axis=0)
        static[name] = jax.device_put(arr, shard0)
    for name, aval in zip(out_names, out_avals):
        z = np.zeros((aval.shape[0] * NCORES,) + tuple(aval.shape[1:]),
                     dtype=aval.dtype)
        static[name] = jax.device_put(z, shard0)

    def run(x_global):
        ops = []
        for name in in_names:
            ops.append(x_global if name == "x" else static[name])
        for name in out_names:
            ops.append(static[name])
        outs = sharded(*ops)
        return dict(zip(out_names, outs))

    _RUNNER = (run, shard0)
    return _RUNNER


def _run_device(x, trace=False):
    """x: [8, 64, 256, 256] fp32. Returns (low, high, results_obj)."""
    if trace:
        from concourse import bass_utils
        nc = build_nc(C_)
        wt = _weights()
        in_maps = [
            dict(wt, x=np.ascontiguousarray(x[b]).reshape(C_, 128, 2, W_))
            for b in range(NCORES)
        ]
        res = bass_utils.run_bass_kernel_spmd(
            nc, in_maps, core_ids=list(range(NCORES)), trace=True)
        low = np.stack([
            res.results[b]["low"].reshape(C_, H_, W_) for b in range(NCORES)])
        high = np.stack([
            res.results[b]["high"].reshape(C_, H_, W_) for b in range(NCORES)])
        return low, high, res

    run, _ = _get_runner()
    outs = run(np.ascontiguousarray(x).reshape(B_ * C_, 128, 2, W_))
    low = np.asarray(outs["low"]).reshape(B_, C_, H_, W_)
    high = np.asarray(outs["high"]).reshape(B_, C_, H_, W_)
    return low, high, None


def _fallback(x, level):
    """Numpy port of the reference for unexpected shapes/levels."""
    xl = x.astype(np.float64)
    low = xl
    high = np.zeros_like(xl)
    Bb, Cc, H, W = xl.shape

    def up(a, n_r, n_c):
        Mr = _resize_matrix(a.shape[-2], n_r)
        Mc = _resize_matrix(a.shape[-1], n_c)
        return np.einsum("ij,...jk,lk->...il", Mr, a, Mc)

    for lv in range(level):
        stride = 2 ** lv
        if H // stride < 2 or W // stride < 2:
            break
        x00 = low[..., 0:H - 1:stride, 0:W - 1:stride]
        x01 = low[..., 0:H - 1:stride, 1:W:stride]
        x10 = low[..., 1:H:stride, 0:W - 1:stride]
        x11 = low[..., 1:H:stride, 1:W:stride]
        ll = (x00 + x01 + x10 + x11) * 0.25
        lh = (x00 + x01 - x10 - x11) * 0.25
        hl = (x00 - x01 + x10 - x11) * 0.25
        hh = (x00 - x01 - x10 + x11) * 0.25
        ch = np.abs(lh) + np.abs(hl) + np.abs(hh)
        high = high + up(ch, H, W)
        low = up(ll, H, W)
    if level > 0:
        high = high / level
    return low.astype(np.float32), high.astype(np.float32)


def kernel(x, level):
    x = np.asarray(x, dtype=np.float32)
    level = int(level)
    if level != 2 or x.shape != (B_, C_, H_, W_):
        return _fallback(x, level)
    low, high, _ = _run_device(x)
    return low, high
